# revision 37
# baseline (speedup 1.0000x reference)
"""Trainium2 Bass kernel for GQA MultiHeadAttention (B=1, S=2048, D=4096,
H=32 query heads, HKV=8 kv heads, DK=DV=128), tensor-parallel across heads
on 8 NeuronCores.

Sharding: core c owns query heads 4c..4c+3 and kv head c (GQA group) and
computes its 512 attention features. The transposed attention output is
AllGathered across cores in four per-q-block collectives (hidden behind
later blocks' compute), then each core computes a 512-row slice of the
transposed output projection.

fp8 acceleration: all four projection GEMMs (Q/K/V/out) run as 3-term
hi/lo float8e4 splits on the PE's DoubleRow mode (2 stacked 128-deep
k-tiles per instruction at 0.5 cycles/col = 4x bf16 throughput):
    x @ W ~= x8@W8 (hi*hi, DoubleRow over chunk pairs)
           + [dW8@x8 + W8@dx8] (one DoubleRow per chunk, lo/hi stacked)
Weights and streamed activations are hi/lo-split and interleaved on the
host; the attention output is split on-device (Act cast + DVE subtract)
before the AllGather. Attention internals (scores/exp/mask/PV/den) stay
bf16: fp8 scores or probabilities fail the 2e-2 gate (measured 4.9e-2),
and raw exp() values overflow fp8's dynamic range.

Scales (powers of 2, exact): activations x16; Wq (with DK^-0.5 folded)
x2^13; Wk/Wv/Wd x2^9. The ones-matrix for the denominator is 512 so the
reciprocal folds the descale and attn comes out x16 ready for fp8; exp
descales scores by 2^-30 via the activation scale; output descales by
2^-13 into bf16 (host upcasts to f32).

Self-contained: hardcodes all shapes; inputs are the full unsharded tensors
keyed as in the problem's setup_inputs().
"""

import numpy as np
import ml_dtypes

import concourse.bacc as bacc
import concourse.mybir as mybir
from concourse.tile import TileContext
from concourse.bass_utils import run_bass_kernel_spmd

BF16 = mybir.dt.bfloat16
F8 = mybir.dt.float8e4
F32 = mybir.dt.float32
DR = mybir.MatmulPerfMode.DoubleRow

N_CORES = 8
S = 2048            # sequence length
D = 4096            # model dim
DK = 128            # head dim
NH_LOC = 4          # query heads per core
FLOC = NH_LOC * DK  # per-core attention features (512)
NDC = D // 128      # contraction chunks of 128 over D (32)
NPR = NDC // 2      # chunk pairs (16)
SB = 512            # q/s block width
NSB = S // SB       # 4
NST = S // 128      # 16 seq tiles of 128

S_A = 16.0          # activation scale
S_WQ = 2.0 ** 13    # Wq scale (DK^-0.5 folded first)
S_WKV = 2.0 ** 9
S_WD = 2.0 ** 9
ONES_VAL = 512.0    # den descale so attn psum*rec = 16*attn
EXP_SCALE = 2.0 ** -30
OUT_DESCALE = 2.0 ** -13

_DMA_TYPES = ("InstDMACopy", "InstDMATranspose")


def _legalize_dma_waits(nc):
    """DMA pseudo-instructions encode at most ONE sem wait (the ISA events
    slot). If Tile's sem assignment leaves more on a DMA, walrus rejects it
    ("Too many sync wait commands"). Hoist all but the last wait onto fresh
    nop instructions inserted immediately before the DMA on the same engine —
    the sequencer executes them in order, so semantics are identical."""
    ctr = 0
    for f in nc.m.functions:
        for blk in f.blocks:
            out = []
            changed = False
            for inst in blk.instructions:
                si = inst.sync_info
                if (
                    si is not None
                    and len(si.on_wait) > 1
                    and type(inst).__name__ in _DMA_TYPES
                ):
                    waits = list(si.on_wait)
                    for w in waits[:-1]:
                        nop = mybir.InstNoOp(
                            name=f"I-dmawaitfix-{ctr}", ins=[], outs=[]
                        )
                        ctr += 1
                        nop.engine = inst.engine
                        nop.sync_info = mybir.SyncInfo(on_wait=[w], on_update=[])
                        out.append(nop)
                    inst.sync_info = mybir.SyncInfo(
                        on_wait=[waits[-1]], on_update=list(si.on_update)
                    )
                    changed = True
                out.append(inst)
            if changed:
                blk.instructions = out
    return ctr


def _build(stage=4, nrep=1):
    nc = bacc.Bacc("TRN2", target_bir_lowering=False, num_devices=N_CORES,
                   dynamic_dma_scratch_size=2048)

    # ---- I/O ----
    # activations: [128, NPR, 2(chunk-in-pair), 2(hi,lo), S] fp8
    qT = nc.dram_tensor("qT", [128, NPR, 2, 2, S], F8, kind="ExternalInput")
    kT = nc.dram_tensor("kT", [128, NPR, 2, 2, S], F8, kind="ExternalInput")
    vT = nc.dram_tensor("vT", [128, NPR, 2, 2, S], F8, kind="ExternalInput")
    # weights: [128, NDC, 2(lo,hi), F] fp8 — hl order REVERSED vs
    # activations so one DoubleRow computes w_lo.T@x_hi + w_hi.T@x_lo
    wqT = nc.dram_tensor("wqT", [128, NDC, 2, FLOC], F8, kind="ExternalInput")
    wkT = nc.dram_tensor("wkT", [128, NDC, 2, DK], F8, kind="ExternalInput")
    wvT = nc.dram_tensor("wvT", [128, NDC, 2, DK], F8, kind="ExternalInput")
    wdT = nc.dram_tensor("wdT", [128, NDC, 2, FLOC], F8, kind="ExternalInput")
    masks = nc.dram_tensor("masks", [128, 4 * SB], BF16, kind="ExternalInput")
    ident = nc.dram_tensor("ident", [128, 128], BF16, kind="ExternalInput")
    outT = nc.dram_tensor("outT", [FLOC, S], BF16, kind="ExternalOutput")

    NBLK = 8
    BLK = NDC // NBLK  # 4 chunks (2 pairs) per Q contraction block
    BPP = BLK // 2     # pairs per block (2)

    with TileContext(nc) as tc:
        with (
            tc.tile_pool(name="consts", bufs=1) as consts,
            tc.tile_pool(name="kvw", bufs=1) as kvw,
            tc.tile_pool(name="bigw", bufs=1) as bigw,
            tc.tile_pool(name="persist", bufs=1) as persist,
            tc.tile_pool(name="qstream", bufs=4) as qstream,
            tc.tile_pool(name="kstream", bufs=2) as kstream,
            tc.tile_pool(name="vstream", bufs=6) as vstream,
            tc.tile_pool(name="erot", bufs=6) as erot,
            tc.tile_pool(name="small", bufs=1) as small,
            tc.tile_pool(name="attnout", bufs=2) as attnout,
            tc.tile_pool(name="abf", bufs=2) as abfp,
            tc.tile_pool(name="atin", bufs=6) as atin,
            tc.tile_pool(name="osb", bufs=2) as osb,
            tc.tile_pool(name="ps", bufs=4, space="PSUM") as ps,
            tc.tile_pool(name="pspv", bufs=2, space="PSUM") as pspv,
            tc.tile_pool(name="psden", bufs=2, space="PSUM") as psden,
            tc.tile_pool(name="dram", bufs=1, space="DRAM") as dram,
        ):
            def one_rep(rep):
                # wk first: K-proj's first matmul waits only on wk + kt[0]
                wk_sb = kvw.tile([128, NDC, 2, DK], F8, name="wk_sb", tag="kvw")
                nc.sync.dma_start(wk_sb[:, 0:BLK, :, :], wkT[:, 0:BLK, :, :])
                ones_sb = consts.tile([128, 128], BF16, name="ones_sb")
                nc.vector.memset(ones_sb[:], ONES_VAL)

                # persistent activations
                QT_sb = persist.tile([128, NH_LOC, S], BF16, name="QT_sb")
                KT_sb = persist.tile([128, S], BF16, name="KT_sb")
                V_sb = persist.tile([128, NST, DK], BF16, name="V_sb")
                QTacc = persist.tile([128, NH_LOC, S], F32, name="QTacc")

                # per-q-block DRAM bounce buffers for the collectives
                # rows: (head, hl, p) so the consumer reads one contiguous
                # 512-row block per fc2 pair
                attn_loc = [
                    dram.tile([2 * FLOC, SB], F8, name=f"attn_loc{qb}", tag=f"al{qb}")
                    for qb in range(NSB)
                ]
                attn_gath = [
                    dram.tile([N_CORES * 2 * FLOC, SB], F8, name=f"attn_gath{qb}",
                              tag=f"ag{qb}", addr_space="Shared")
                    for qb in range(NSB)
                ]

                # ---- front: K-proj and Q-proj interleaved at pair level ----
                q_pairs = {}
                wq_holder = []

                # Q contraction blocks as pair-lists: a 1-pair block 0 (its
                # units start right after qt0 — fills the DMA-bound warmup)
                # and a 1-pair block 8 at the end (short units that defer
                # cheaply into V phase A).
                PAIR_BLOCKS = ([[0]] + [[2 * i + 1, 2 * i + 2]
                                        for i in range(7)] + [[15]])
                NQB = len(PAIR_BLOCKS)  # 9

                def emit_q_unit(b, f, pair):
                    # one head-feature (f) x one sb-pair of contraction block b
                    tpool, ttag = (pspv, "pv") if pair == 0 else (psden, "den")
                    qp = [
                        tpool.tile([128, SB], F32, name=f"qp{j}", tag=ttag)
                        for j in range(2)
                    ]
                    prs = PAIR_BLOCKS[b]
                    for pj, pr in enumerate(prs):
                        dc0 = 2 * pr
                        qt = q_pairs[pr]
                        for j2 in range(2):
                            s2 = 2 * pair + j2
                            cols = slice(s2 * SB, (s2 + 1) * SB)
                            fcols = slice(f * 128, (f + 1) * 128)
                            nc.tensor.matmul(
                                qp[j2][:],
                                lhsT=wq_holder[0][:, dc0:dc0 + 2, 1, fcols],
                                rhs=qt[:, :, 0, cols],
                                start=(pj == 0), stop=False, perf_mode=DR,
                            )
                            for j in range(2):
                                nc.tensor.matmul(
                                    qp[j2][:],
                                    lhsT=wq_holder[0][:, dc0 + j, :, fcols],
                                    rhs=qt[:, j, :, cols],
                                    start=False,
                                    stop=(pj == len(prs) - 1 and j == 1),
                                    perf_mode=DR,
                                )
                    for j in range(2):
                        s2 = 2 * pair + j
                        dst_acc = QTacc[:, f, s2 * SB:(s2 + 1) * SB]
                        if b == 0:
                            nc.vector.tensor_copy(dst_acc, qp[j][:])
                        elif b < NQB - 1:
                            nc.vector.tensor_tensor(
                                dst_acc, dst_acc, qp[j][:], mybir.AluOpType.add
                            )
                        else:
                            nc.vector.tensor_tensor(
                                QT_sb[:, f, s2 * SB:(s2 + 1) * SB],
                                dst_acc, qp[j][:], mybir.AluOpType.add,
                            )

                def proj_pair(ps_tiles, w_sb, x_t, pr, sbs):
                    # 3-term hi/lo DoubleRow for one chunk pair
                    dc0 = 2 * pr
                    for sb in sbs:
                        cols = slice(sb * SB, (sb + 1) * SB)
                        nc.tensor.matmul(
                            ps_tiles[sb][:],
                            lhsT=w_sb[:, dc0:dc0 + 2, 1, :],
                            rhs=x_t[:, :, 0, cols],
                            start=(pr == 0), stop=False, perf_mode=DR,
                        )
                        for j in range(2):
                            nc.tensor.matmul(
                                ps_tiles[sb][:],
                                lhsT=w_sb[:, dc0 + j, :, :],
                                rhs=x_t[:, j, :, cols],
                                start=False,
                                stop=(pr == NPR - 1 and j == 1),
                                perf_mode=DR,
                            )

                k_ps = [ps.tile([128, SB], F32, name=f"kps{i}", tag="ps")
                        for i in range(NSB)]
                # F emits blocks 0..7 minus block 7's heads 2,3 (those 4 and
                # all of block 8 defer into V phase A). Availability: block b
                # is runnable once its last qt pair (pr = 2b for b>=1, pr 0
                # for b0) and wq pair-slices have landed.
                f_units = [(b, f, pair) for b in range(NQB - 1)
                           for f in range(NH_LOC) for pair in range(2)][:-4]
                fui = 0
                for pr in range(NPR):
                    kt_t = kstream.tile([128, 2, 2, S], F8, name="kt_t",
                                        tag="kt")
                    nc.sync.dma_start(kt_t[:], kT[:, pr, :, :, :])
                    qt_t = qstream.tile([128, 2, 2, S], F8, name="qt_t",
                                        tag="qt")
                    nc.sync.dma_start(qt_t[:], qT[:, pr, :, :, :])
                    q_pairs[pr] = qt_t
                    if pr == 0:
                        wq_sb = bigw.tile([128, NDC, 2, FLOC], F8,
                                          name="wq_sb", tag="bigw")
                        wq_holder.append(wq_sb)
                    if pr % BPP == 0 and pr > 0:
                        dc0 = 2 * pr
                        nc.sync.dma_start(
                            wk_sb[:, dc0:dc0 + BLK, :, :],
                            wkT[:, dc0:dc0 + BLK, :, :])
                    # wq pair-slice (small, so block 0's units unblock early)
                    nc.sync.dma_start(
                        wq_sb[:, 2 * pr:2 * pr + 2, :, :],
                        wqT[:, 2 * pr:2 * pr + 2, :, :])
                    # units BEFORE this pair's K-proj: by the time the PE
                    # reaches them their qt pairs have arrived, while K-proj
                    # waits on this pair's kt DMA — in-order PE
                    avail = 8 * (1 + pr // 2)
                    target = min(len(f_units), avail, 4 + 4 * pr)
                    while fui < target:
                        emit_q_unit(*f_units[fui])
                        fui += 1
                    proj_pair(k_ps, wk_sb, kt_t, pr, range(NSB))
                for sb in range(NSB):
                    nc.vector.tensor_copy(KT_sb[:, sb * SB:(sb + 1) * SB], k_ps[sb][:])

                # ---- V projection in column halves so k-tiles 0..7 are
                # ---- ready at half-stream: Q's last block weaves into half 1
                # ---- (PE-idle DMA windows), attention qb0/qb1 into half 2.
                wv_sb = kvw.tile([128, NDC, 2, DK], F8, name="wv_sb", tag="kvw")
                nc.sync.dma_start(wv_sb[:, 0:BLK, :, :], wvT[:, 0:BLK, :, :])
                ident_sb = consts.tile([128, 128], BF16, name="ident_sb")
                masks_sb = consts.tile([128, 4 * SB], BF16, name="masks_sb")

                VT_sb = persist.tile([128, S], BF16, name="VT_sb")

                # --- attention machinery (emitted incrementally) ---
                hilo_tiles = {}
                wd_holder = []

                def attn_head(qb, h, mid=None):
                    # mid: emitted after the first score_exp — V-transpose
                    # groups slot here so their cross-engine latency hides
                    # behind this head's remaining scores
                    nkt = 4 * qb + 4  # causal: k-tiles 0..4qb+3
                    if qb not in hilo_tiles:
                        hilo_tiles[qb] = attnout.tile(
                            [128, NH_LOC, 2, SB], F8, name="hilo", tag="attn")
                    hilo = hilo_tiles[qb]
                    den_ps = psden.tile([128, SB], F32, name="den_ps", tag="den")
                    att_ps = pspv.tile([128, SB], F32, name="att_ps", tag="pv")
                    split = nkt <= 4
                    E_tiles = []

                    def score_exp(kt):
                        d = kt - 4 * qb
                        off = 128 * d if d >= 1 else 0
                        st_ps = ps.tile([128, SB], F32, name="st_ps", tag="ps")
                        nc.tensor.matmul(
                            st_ps[:, off:],
                            lhsT=KT_sb[:, kt * 128:(kt + 1) * 128],
                            rhs=QT_sb[:, h, qb * SB + off:(qb + 1) * SB],
                            start=True,
                            stop=True,
                        )
                        E1 = erot.tile([128, SB], BF16, name="E1", tag="E")
                        nc.scalar.activation(
                            E1[:, off:], st_ps[:, off:],
                            mybir.ActivationFunctionType.Exp,
                            scale=EXP_SCALE,
                        )
                        if d >= 0:
                            nc.vector.tensor_tensor(
                                E1[:, off:],
                                E1[:, off:],
                                masks_sb[:, d * SB + off:(d + 1) * SB],
                                mybir.AluOpType.mult,
                            )
                        return E1

                    def den_pv(kt, E1):
                        d = kt - 4 * qb
                        off = 128 * d if d >= 1 else 0
                        nc.tensor.matmul(
                            den_ps[:, off:],
                            lhsT=ones_sb[:, :],
                            rhs=E1[:, off:],
                            start=(kt == 0),
                            stop=(kt == nkt - 1),
                        )
                        nc.tensor.matmul(
                            att_ps[:, off:],
                            lhsT=V_sb[:, kt, :],
                            rhs=E1[:, off:],
                            start=(kt == 0),
                            stop=(kt == nkt - 1),
                        )

                    if split:
                        for kt in range(nkt):
                            E_tiles.append(score_exp(kt))
                            if kt == 0 and mid is not None:
                                mid()
                        for kt in range(nkt):
                            den_pv(kt, E_tiles[kt])
                    else:
                        for kt in range(nkt):
                            E1 = score_exp(kt)
                            if kt == 0 and mid is not None:
                                mid()
                            den_pv(kt, E1)
                    # normalize + split to fp8 hi/lo (attn scaled x16)
                    rec = small.tile([128, SB], F32, name="rec", tag="rec")
                    nc.vector.reciprocal(rec[:], den_ps[:])
                    abf = abfp.tile([128, SB], BF16, name="abf", tag="abf")
                    nc.vector.tensor_tensor(
                        abf[:], att_ps[:], rec[:], mybir.AluOpType.mult
                    )
                    nc.scalar.copy(hilo[:, h, 0, :], abf[:])
                    nc.vector.tensor_tensor(
                        hilo[:, h, 1, :], abf[:], hilo[:, h, 0, :],
                        mybir.AluOpType.subtract,
                    )
                    nc.sync.dma_start(
                        attn_loc[qb][h * 256:h * 256 + 256, :]
                        .rearrange("(hl p) q -> p hl q", p=128),
                        hilo[:, h, :, :],
                    )

                def attn_gather(qb):
                    if stage >= 4:
                        nc.gpsimd.collective_compute(
                            "AllGather",
                            mybir.AluOpType.bypass,
                            replica_groups=[list(range(N_CORES))],
                            ins=[attn_loc[qb][:]],
                            outs=[attn_gath[qb][:]],
                        )

                def wd_slice(qb):
                    if qb == 0:
                        wd_holder.append(bigw.tile(
                            [128, NDC, 2, FLOC], F8, name="wd_sb", tag="bigw"))
                    wdc = NDC // NSB
                    nc.sync.dma_start(
                        wd_holder[0][:, wdc * qb:wdc * (qb + 1), :, :],
                        wdT[:, wdc * qb:wdc * (qb + 1), :, :])

                def transp_group(g, pool, tag):
                    tp = pool.tile([128, 4, 128], BF16, name="tp", tag=tag)
                    for j in range(4):
                        st = 4 * g + j
                        nc.tensor.transpose(
                            tp[:, j, :], VT_sb[:, st * 128:(st + 1) * 128],
                            ident_sb[:])
                    nc.vector.tensor_copy(V_sb[:, 4 * g:4 * (g + 1), :], tp[:])

                # --- V phase A (cols 0:512, sb0) — 12 Q units run here:
                # block 7's 8 plus block 6's deferred 4 (heads 2,3). Units are
                # emitted before the V-proj matmuls: their inputs (qt pairs)
                # land before vt does, so they fill the F-tail DMA window.
                # b6 units first (earliest-arriving qt, and each head's QTacc
                # chain must run b6 before b7).
                h1_units = [(NQB - 2, 2, 0), (NQB - 2, 2, 1),
                            (NQB - 2, 3, 0), (NQB - 2, 3, 1),
                            (NQB - 1, 0, 0), (NQB - 1, 0, 1),
                            (NQB - 1, 1, 0), (NQB - 1, 1, 1),
                            (NQB - 1, 2, 0), (NQB - 1, 2, 1),
                            (NQB - 1, 3, 0), (NQB - 1, 3, 1)]
                attn_jobs = [(0, 0), (0, 1), (0, 2), (0, 3),
                             (1, 0), (1, 1), (1, 2), (1, 3)]
                emitted = 0

                def pump_attn(n, mid=None):
                    nonlocal emitted
                    for _ in range(n):
                        if emitted >= len(attn_jobs):
                            if mid is not None:
                                mid()
                            return
                        qb, h = attn_jobs[emitted]
                        attn_head(qb, h, mid=mid)
                        mid = None
                        emitted += 1
                        if qb == 0 and h == NH_LOC - 1:
                            attn_gather(0)
                        if qb == 1 and h == NH_LOC - 1:
                            attn_gather(1)

                v_psA = ps.tile([128, SB], F32, name="v_psA", tag="ps")
                ui = 0
                for pr in range(NPR):
                    if pr % BPP == 1 and pr < NPR - BPP:
                        j = pr // BPP + 1
                        nc.sync.dma_start(
                            wv_sb[:, BLK * j:BLK * (j + 1), :, :],
                            wvT[:, BLK * j:BLK * (j + 1), :, :])
                    vt_t = vstream.tile([128, 2, 2, SB], F8,
                                        name="vt_t", tag="vt")
                    nc.sync.dma_start(vt_t[:], vT[:, pr, :, :, 0:SB])
                    if pr == 0:
                        while ui < 4:
                            emit_q_unit(*h1_units[ui])
                            ui += 1
                    elif ui < len(h1_units):
                        emit_q_unit(*h1_units[ui])
                        ui += 1
                    proj_pair([v_psA], wv_sb, vt_t, pr, [0])
                while ui < len(h1_units):
                    emit_q_unit(*h1_units[ui])
                    ui += 1
                nc.vector.tensor_copy(VT_sb[:, 0:SB], v_psA[:])
                nc.sync.dma_start(ident_sb[:], ident[:])
                nc.sync.dma_start(masks_sb[:], masks[:])

                # --- V phase B (cols 512:1024, sb1) with qb0 woven ---
                v_psB = ps.tile([128, SB], F32, name="v_psB", tag="ps")
                for pr in range(NPR):
                    vt_t = vstream.tile([128, 2, 2, SB], F8,
                                        name="vt_t", tag="vt")
                    nc.sync.dma_start(vt_t[:], vT[:, pr, :, :, SB:2 * SB])
                    if stage >= 2 and pr % 4 == 0:
                        pump_attn(1, mid=(
                            (lambda: transp_group(0, pspv, "pv"))
                            if pr == 0 else None))
                    proj_pair([v_psB], wv_sb, vt_t, pr, [0])
                nc.vector.tensor_copy(VT_sb[:, SB:2 * SB], v_psB[:])

                # --- V phase C (cols 1024:2048, sb2+sb3) with qb1 woven ---
                v_psC = [ps.tile([128, SB], F32, name=f"v_psC{i}", tag="ps")
                         for i in range(2)]
                for pr in range(NPR):
                    vt_t = vstream.tile([128, 2, 2, 2 * SB], F8,
                                        name="vt_t", tag="vt")
                    nc.sync.dma_start(vt_t[:], vT[:, pr, :, :, 2 * SB:4 * SB])
                    if stage >= 2 and pr % 4 == 0:
                        pump_attn(1, mid=(
                            (lambda: transp_group(1, psden, "den"))
                            if pr == 0 else None))
                    proj_pair(v_psC, wv_sb, vt_t, pr, range(2))
                for sb in range(2):
                    nc.vector.tensor_copy(VT_sb[:, (2 + sb) * SB:(3 + sb) * SB],
                                          v_psC[sb][:])
                for g in range(2, 4):
                    transp_group(g, ps, "ps")

                # ---- rest of attention (qb1 remainder, qb2, qb3) ----
                if stage >= 2:
                    pump_attn(len(attn_jobs) - emitted)
                    wd_slice(0)
                    wd_slice(1)
                    for qb in range(2, NSB):
                        for h in range(NH_LOC):
                            attn_head(qb, h)
                        attn_gather(qb)
                        wd_slice(qb)

                # ---- output projection per q-block ----
                if stage >= 4:
                    def op_block(qb, dsubs):
                        o_ps = [
                            ps.tile([128, SB], F32, name=f"ops{d2}", tag="ps")
                            if d2 < 2 else
                            (pspv.tile([128, SB], F32, name=f"ops{d2}", tag="pv")
                             if d2 == 2 else
                             psden.tile([128, SB], F32, name=f"ops{d2}", tag="den"))
                            for d2 in dsubs
                        ]
                        for fc2 in range(NDC // 2):
                            at = atin.tile([128, 2, 2, SB], F8, name="at_c",
                                           tag="atin")
                            rowstart = fc2 * 512
                            nc.sync.dma_start(
                                at[:],
                                attn_gath[qb][rowstart:rowstart + 512, :]
                                .rearrange("(j hl p) q -> p j hl q", p=128, hl=2),
                            )
                            for j, dsub in enumerate(dsubs):
                                fcols = slice(dsub * 128, (dsub + 1) * 128)
                                nc.tensor.matmul(
                                    o_ps[j][:],
                                    lhsT=wd_holder[0][:, 2 * fc2:2 * fc2 + 2, 1, fcols],
                                    rhs=at[:, :, 0, :],
                                    start=(fc2 == 0), stop=False, perf_mode=DR,
                                )
                                for j2 in range(2):
                                    nc.tensor.matmul(
                                        o_ps[j][:],
                                        lhsT=wd_holder[0][:, 2 * fc2 + j2, :, fcols],
                                        rhs=at[:, j2, :, :],
                                        start=False,
                                        stop=(fc2 == NDC // 2 - 1 and j2 == 1),
                                        perf_mode=DR,
                                    )
                        # descale into one tile; two DMAs so the first pair's
                        # writeback overlaps the second pair's descale
                        o_all = osb.tile([128, 4, SB], BF16, name="o_all",
                                         tag="osb")
                        for j, dsub in enumerate(dsubs):
                            if j % 2 == 0:
                                nc.vector.tensor_scalar_mul(
                                    o_all[:, dsub, :], o_ps[j][:], OUT_DESCALE)
                            else:
                                nc.scalar.mul(
                                    o_all[:, dsub, :], o_ps[j][:], OUT_DESCALE)
                            if j == 1:
                                nc.sync.dma_start(
                                    outT[0:2 * 128, qb * SB:(qb + 1) * SB]
                                    .rearrange("(j p) q -> p j q", p=128),
                                    o_all[:, 0:2, :],
                                )
                        nc.sync.dma_start(
                            outT[2 * 128:FLOC, qb * SB:(qb + 1) * SB]
                            .rearrange("(j p) q -> p j q", p=128),
                            o_all[:, 2:4, :],
                        )

                    for qb in range(NSB):
                        op_block(qb, [0, 1, 2, 3])

            for rep in range(nrep):
                one_rep(rep)

    nc.compile()
    _legalize_dma_waits(nc)
    nc.codegen_inst_isa_subclasses()
    return nc


_NC_CACHE = None


def _get_nc():
    global _NC_CACHE
    if _NC_CACHE is None:
        _NC_CACHE = _build()
    return _NC_CACHE


def _split8(x, scale):
    """x (f32 [D, S]) -> hi, lo fp8 arrays of the scaled value."""
    f8 = ml_dtypes.float8_e4m3
    xs = x * np.float32(scale)
    hi = xs.astype(f8)
    lo = (xs - hi.astype(np.float32)).astype(f8)
    return hi, lo


def _act_layout(xT_hi, xT_lo):
    """[D, S] fp8 pair -> [128, NPR, 2, 2, S]."""
    h = xT_hi.reshape(NPR, 2, 128, S)
    l = xT_lo.reshape(NPR, 2, 128, S)
    out = np.empty((128, NPR, 2, 2, S), dtype=xT_hi.dtype)
    out[:, :, :, 0, :] = h.transpose(2, 0, 1, 3)
    out[:, :, :, 1, :] = l.transpose(2, 0, 1, 3)
    return np.ascontiguousarray(out)


def _w_layout(wT_hi, wT_lo):
    """[D, F] fp8 pair -> [128, NDC, 2(lo,hi), F]."""
    f = wT_hi.shape[1]
    h = wT_hi.reshape(NDC, 128, f)
    l = wT_lo.reshape(NDC, 128, f)
    out = np.empty((128, NDC, 2, f), dtype=wT_hi.dtype)
    out[:, :, 0, :] = l.transpose(1, 0, 2)
    out[:, :, 1, :] = h.transpose(1, 0, 2)
    return np.ascontiguousarray(out)


def _make_in_maps(q, k, v, Wq, Wk, Wv, Wd):
    bf = ml_dtypes.bfloat16
    scale = np.float32(DK) ** -0.5
    qT = np.ascontiguousarray(q.reshape(S, D).T)
    kT = np.ascontiguousarray(k.reshape(S, D).T)
    vT = np.ascontiguousarray(v.reshape(S, D).T)
    qA = _act_layout(*_split8(qT, S_A))
    kA = _act_layout(*_split8(kT, S_A))
    vA = _act_layout(*_split8(vT, S_A))

    kp = np.arange(128, dtype=np.int32)[:, None]
    qf = np.arange(SB, dtype=np.int32)[None, :]
    masks = np.concatenate(
        [(qf >= kp + 128 * d).astype(np.float32) for d in range(4)], axis=1
    ).astype(bf)
    ident = np.eye(128, dtype=np.float32).astype(bf)

    in_maps = []
    for c in range(N_CORES):
        fs = slice(FLOC * c, FLOC * (c + 1))
        ks = slice(DK * c, DK * (c + 1))
        in_maps.append({
            "qT": qA,
            "kT": kA,
            "vT": vA,
            "wqT": _w_layout(*_split8((Wq[fs, :] * scale).T, S_WQ)),
            "wkT": _w_layout(*_split8(Wk[ks, :].T, S_WKV)),
            "wvT": _w_layout(*_split8(Wv[ks, :].T, S_WKV)),
            "wdT": _w_layout(*_split8(Wd[fs, :].T, S_WD)),
            "masks": masks,
            "ident": ident,
        })
    return in_maps


def _assemble(results):
    outT_full = np.concatenate(
        [r["outT"].astype(np.float32) for r in results], axis=0)  # [4096, 2048]
    return np.ascontiguousarray(outT_full.T).reshape(1, S, D).astype(np.float32)


def kernel(q, k, v, Wq, Wk, Wv, Wd, _trace=False, **_ignored):
    nc = _get_nc()
    in_maps = _make_in_maps(
        np.asarray(q, np.float32), np.asarray(k, np.float32),
        np.asarray(v, np.float32), np.asarray(Wq, np.float32),
        np.asarray(Wk, np.float32), np.asarray(Wv, np.float32),
        np.asarray(Wd, np.float32),
    )
    res = run_bass_kernel_spmd(
        nc, in_maps, core_ids=list(range(N_CORES)), trace=_trace
    )
    out = _assemble(res.results)
    if _trace:
        return out, res
    return out


# revision 46
# speedup vs baseline: 42903.4581x; 42903.4581x over previous
"""Trainium2 Bass kernel for GQA MultiHeadAttention (B=1, S=2048, D=4096,
H=32 query heads, HKV=8 kv heads, DK=DV=128), tensor-parallel across heads
on 8 NeuronCores.

Sharding: core c owns query heads 4c..4c+3 and kv head c (GQA group) and
computes its 512 attention features. The transposed attention output is
AllGathered across cores in four per-q-block collectives (hidden behind
later blocks' compute), then each core computes a 512-row slice of the
transposed output projection.

fp8 acceleration: all four projection GEMMs (Q/K/V/out) run as 3-term
hi/lo float8e4 splits on the PE's DoubleRow mode (2 stacked 128-deep
k-tiles per instruction at 0.5 cycles/col = 4x bf16 throughput):
    x @ W ~= x8@W8 (hi*hi, DoubleRow over chunk pairs)
           + [dW8@x8 + W8@dx8] (one DoubleRow per chunk, lo/hi stacked)
Weights and streamed activations are hi/lo-split and interleaved on the
host; the attention output is split on-device (Act cast + DVE subtract)
before the AllGather. Attention internals (scores/exp/mask/PV/den) stay
bf16: fp8 scores or probabilities fail the 2e-2 gate (measured 4.9e-2),
and raw exp() values overflow fp8's dynamic range.

Scales (powers of 2, exact): activations x16; Wq (with DK^-0.5 folded)
x2^13; Wk/Wv/Wd x2^9. The ones-matrix for the denominator is 512 so the
reciprocal folds the descale and attn comes out x16 ready for fp8; exp
descales scores by 2^-30 via the activation scale; output descales by
2^-13 into bf16 (host upcasts to f32).

Self-contained: hardcodes all shapes; inputs are the full unsharded tensors
keyed as in the problem's setup_inputs().
"""

import numpy as np
import ml_dtypes

import concourse.bacc as bacc
import concourse.mybir as mybir
from concourse.tile import TileContext
from concourse.bass_utils import run_bass_kernel_spmd

BF16 = mybir.dt.bfloat16
F8 = mybir.dt.float8e4
F32 = mybir.dt.float32
DR = mybir.MatmulPerfMode.DoubleRow

N_CORES = 8
S = 2048            # sequence length
D = 4096            # model dim
DK = 128            # head dim
NH_LOC = 4          # query heads per core
FLOC = NH_LOC * DK  # per-core attention features (512)
NDC = D // 128      # contraction chunks of 128 over D (32)
NPR = NDC // 2      # chunk pairs (16)
SB = 512            # q/s block width
NSB = S // SB       # 4
NST = S // 128      # 16 seq tiles of 128

S_A = 16.0          # activation scale
S_WQ = 2.0 ** 13    # Wq scale (DK^-0.5 folded first)
S_WKV = 2.0 ** 9
S_WD = 2.0 ** 9
ONES_VAL = 512.0    # den descale so attn psum*rec = 16*attn
EXP_SCALE = 2.0 ** -30
OUT_DESCALE = 2.0 ** -13

_DMA_TYPES = ("InstDMACopy", "InstDMATranspose")


def _legalize_dma_waits(nc):
    """DMA pseudo-instructions encode at most ONE sem wait (the ISA events
    slot). If Tile's sem assignment leaves more on a DMA, walrus rejects it
    ("Too many sync wait commands"). Hoist all but the last wait onto fresh
    nop instructions inserted immediately before the DMA on the same engine —
    the sequencer executes them in order, so semantics are identical."""
    ctr = 0
    for f in nc.m.functions:
        for blk in f.blocks:
            out = []
            changed = False
            for inst in blk.instructions:
                si = inst.sync_info
                if (
                    si is not None
                    and len(si.on_wait) > 1
                    and type(inst).__name__ in _DMA_TYPES
                ):
                    waits = list(si.on_wait)
                    for w in waits[:-1]:
                        nop = mybir.InstNoOp(
                            name=f"I-dmawaitfix-{ctr}", ins=[], outs=[]
                        )
                        ctr += 1
                        nop.engine = inst.engine
                        nop.sync_info = mybir.SyncInfo(on_wait=[w], on_update=[])
                        out.append(nop)
                    inst.sync_info = mybir.SyncInfo(
                        on_wait=[waits[-1]], on_update=list(si.on_update)
                    )
                    changed = True
                out.append(inst)
            if changed:
                blk.instructions = out
    return ctr


def _build(stage=4, nrep=1):
    nc = bacc.Bacc("TRN2", target_bir_lowering=False, num_devices=N_CORES,
                   dynamic_dma_scratch_size=2048)

    # ---- I/O ----
    # activations: [128, NPR, 2(chunk-in-pair), 2(hi,lo), S] fp8
    qT = nc.dram_tensor("qT", [128, NPR, 2, 2, S], F8, kind="ExternalInput")
    kT = nc.dram_tensor("kT", [128, NPR, 2, 2, S], F8, kind="ExternalInput")
    vT = nc.dram_tensor("vT", [128, NPR, 2, 2, S], F8, kind="ExternalInput")
    # weights: [128, NDC, 2(lo,hi), F] fp8 — hl order REVERSED vs
    # activations so one DoubleRow computes w_lo.T@x_hi + w_hi.T@x_lo
    wqT = nc.dram_tensor("wqT", [128, NDC, 2, FLOC], F8, kind="ExternalInput")
    wkT = nc.dram_tensor("wkT", [128, NDC, 2, DK], F8, kind="ExternalInput")
    wvT = nc.dram_tensor("wvT", [128, NDC, 2, DK], F8, kind="ExternalInput")
    wdT = nc.dram_tensor("wdT", [128, NDC, 2, FLOC], F8, kind="ExternalInput")
    masks = nc.dram_tensor("masks", [128, 4 * SB], BF16, kind="ExternalInput")
    ident = nc.dram_tensor("ident", [128, 128], BF16, kind="ExternalInput")
    outT = nc.dram_tensor("outT", [FLOC, S], BF16, kind="ExternalOutput")

    NBLK = 8
    BLK = NDC // NBLK  # 4 chunks (2 pairs) per Q contraction block
    BPP = BLK // 2     # pairs per block (2)

    with TileContext(nc) as tc:
        with (
            tc.tile_pool(name="consts", bufs=1) as consts,
            tc.tile_pool(name="kvw", bufs=1) as kvw,
            tc.tile_pool(name="bigw", bufs=1) as bigw,
            tc.tile_pool(name="persist", bufs=1) as persist,
            tc.tile_pool(name="qstream", bufs=4) as qstream,
            tc.tile_pool(name="kstream", bufs=3) as kstream,
            tc.tile_pool(name="vstream", bufs=6) as vstream,
            tc.tile_pool(name="erot", bufs=6) as erot,
            tc.tile_pool(name="small", bufs=1) as small,
            tc.tile_pool(name="attnout", bufs=2) as attnout,
            tc.tile_pool(name="abf", bufs=2) as abfp,
            tc.tile_pool(name="atin", bufs=6) as atin,
            tc.tile_pool(name="osb", bufs=2) as osb,
            tc.tile_pool(name="ps", bufs=4, space="PSUM") as ps,
            tc.tile_pool(name="pspv", bufs=2, space="PSUM") as pspv,
            tc.tile_pool(name="psden", bufs=2, space="PSUM") as psden,
            tc.tile_pool(name="dram", bufs=1, space="DRAM") as dram,
        ):
            def one_rep(rep):
                # wk first: K-proj's first matmul waits only on wk + kt[0]
                wk_sb = kvw.tile([128, NDC, 2, DK], F8, name="wk_sb", tag="kvw")
                nc.sync.dma_start(wk_sb[:, 0:BLK, :, :], wkT[:, 0:BLK, :, :])
                ones_sb = consts.tile([128, 128], BF16, name="ones_sb")
                nc.vector.memset(ones_sb[:], ONES_VAL)

                # persistent activations
                QT_sb = persist.tile([128, NH_LOC, S], BF16, name="QT_sb")
                KT_sb = persist.tile([128, S], BF16, name="KT_sb")
                V_sb = persist.tile([128, NST, DK], BF16, name="V_sb")
                QTacc = persist.tile([128, NH_LOC, S], BF16, name="QTacc")

                # per-q-block DRAM bounce buffers for the collectives
                # rows: (head, hl, p) so the consumer reads one contiguous
                # 512-row block per fc2 pair
                attn_loc = [
                    dram.tile([2 * FLOC, SB], F8, name=f"attn_loc{qb}", tag=f"al{qb}")
                    for qb in range(NSB)
                ]
                attn_gath = [
                    dram.tile([N_CORES * 2 * FLOC, SB], F8, name=f"attn_gath{qb}",
                              tag=f"ag{qb}", addr_space="Shared")
                    for qb in range(NSB)
                ]

                # ---- front: K-proj and Q-proj interleaved at pair level ----
                q_pairs = {}
                wq_holder = []

                # Q contraction blocks as pair-lists: a 1-pair block 0 (its
                # units start right after qt0 — fills the DMA-bound warmup)
                # and a 1-pair block 8 at the end (short units that defer
                # cheaply into V phase A).
                PAIR_BLOCKS = ([[0]] + [[2 * i + 1, 2 * i + 2]
                                        for i in range(7)] + [[15]])
                NQB = len(PAIR_BLOCKS)  # 9

                def emit_q_unit(b, f, pair):
                    # one head-feature (f) x one sb-pair of contraction block b
                    tpool, ttag = (pspv, "pv") if pair == 0 else (psden, "den")
                    qp = [
                        tpool.tile([128, SB], F32, name=f"qp{j}", tag=ttag)
                        for j in range(2)
                    ]
                    prs = PAIR_BLOCKS[b]
                    for pj, pr in enumerate(prs):
                        dc0 = 2 * pr
                        qt = q_pairs[pr]
                        for j2 in range(2):
                            s2 = 2 * pair + j2
                            cols = slice(s2 * SB, (s2 + 1) * SB)
                            fcols = slice(f * 128, (f + 1) * 128)
                            nc.tensor.matmul(
                                qp[j2][:],
                                lhsT=wq_holder[0][:, dc0:dc0 + 2, 1, fcols],
                                rhs=qt[:, :, 0, cols],
                                start=(pj == 0), stop=False, perf_mode=DR,
                            )
                            for j in range(2):
                                nc.tensor.matmul(
                                    qp[j2][:],
                                    lhsT=wq_holder[0][:, dc0 + j, :, fcols],
                                    rhs=qt[:, j, :, cols],
                                    start=False,
                                    stop=(pj == len(prs) - 1 and j == 1),
                                    perf_mode=DR,
                                )
                    for j in range(2):
                        s2 = 2 * pair + j
                        dst_acc = QTacc[:, f, s2 * SB:(s2 + 1) * SB]
                        if b == 0:
                            nc.vector.tensor_copy(dst_acc, qp[j][:])
                        elif b < NQB - 1:
                            nc.vector.tensor_tensor(
                                dst_acc, dst_acc, qp[j][:], mybir.AluOpType.add
                            )
                        else:
                            nc.vector.tensor_tensor(
                                QT_sb[:, f, s2 * SB:(s2 + 1) * SB],
                                dst_acc, qp[j][:], mybir.AluOpType.add,
                            )

                def proj_pair(ps_tiles, w_sb, x_t, pr, sbs):
                    # 3-term hi/lo DoubleRow for one chunk pair
                    dc0 = 2 * pr
                    for sb in sbs:
                        cols = slice(sb * SB, (sb + 1) * SB)
                        nc.tensor.matmul(
                            ps_tiles[sb][:],
                            lhsT=w_sb[:, dc0:dc0 + 2, 1, :],
                            rhs=x_t[:, :, 0, cols],
                            start=(pr == 0), stop=False, perf_mode=DR,
                        )
                        for j in range(2):
                            nc.tensor.matmul(
                                ps_tiles[sb][:],
                                lhsT=w_sb[:, dc0 + j, :, :],
                                rhs=x_t[:, j, :, cols],
                                start=False,
                                stop=(pr == NPR - 1 and j == 1),
                                perf_mode=DR,
                            )

                k_ps = [ps.tile([128, SB], F32, name=f"kps{i}", tag="ps")
                        for i in range(NSB)]
                # F emits blocks 0..7 minus block 7's heads 2,3 (those 4 and
                # all of block 8 defer into V phase A). Availability: block b
                # is runnable once its last qt pair (pr = 2b for b>=1, pr 0
                # for b0) and wq pair-slices have landed.
                f_units = [(b, f, pair) for b in range(NQB - 1)
                           for f in range(NH_LOC) for pair in range(2)][:-4]
                fui = 0
                for pr in range(NPR):
                    kt_t = kstream.tile([128, 2, 2, S], F8, name="kt_t",
                                        tag="kt")
                    nc.sync.dma_start(kt_t[:], kT[:, pr, :, :, :])
                    qt_t = qstream.tile([128, 2, 2, S], F8, name="qt_t",
                                        tag="qt")
                    nc.sync.dma_start(qt_t[:], qT[:, pr, :, :, :])
                    q_pairs[pr] = qt_t
                    if pr == 0:
                        wq_sb = bigw.tile([128, NDC, 2, FLOC], F8,
                                          name="wq_sb", tag="bigw")
                        wq_holder.append(wq_sb)
                    if pr % BPP == 0 and pr > 0:
                        dc0 = 2 * pr
                        nc.sync.dma_start(
                            wk_sb[:, dc0:dc0 + BLK, :, :],
                            wkT[:, dc0:dc0 + BLK, :, :])
                    # wq pair-slice (small, so block 0's units unblock early)
                    nc.sync.dma_start(
                        wq_sb[:, 2 * pr:2 * pr + 2, :, :],
                        wqT[:, 2 * pr:2 * pr + 2, :, :])
                    # units BEFORE this pair's K-proj: by the time the PE
                    # reaches them their qt pairs have arrived, while K-proj
                    # waits on this pair's kt DMA — in-order PE. Exception:
                    # at pr0, kt0 lands before qt0, so K-proj goes first.
                    avail = 8 * (1 + pr // 2)
                    target = min(len(f_units), avail, 4 + 4 * pr)
                    if pr == 0:
                        proj_pair(k_ps, wk_sb, kt_t, pr, range(NSB))
                    while fui < target:
                        emit_q_unit(*f_units[fui])
                        fui += 1
                    if pr > 0:
                        proj_pair(k_ps, wk_sb, kt_t, pr, range(NSB))
                for sb in range(NSB):
                    nc.vector.tensor_copy(KT_sb[:, sb * SB:(sb + 1) * SB], k_ps[sb][:])

                # ---- V projection in column halves so k-tiles 0..7 are
                # ---- ready at half-stream: Q's last block weaves into half 1
                # ---- (PE-idle DMA windows), attention qb0/qb1 into half 2.
                wv_sb = kvw.tile([128, NDC, 2, DK], F8, name="wv_sb", tag="kvw")
                nc.sync.dma_start(wv_sb[:, 0:BLK, :, :], wvT[:, 0:BLK, :, :])
                ident_sb = consts.tile([128, 128], BF16, name="ident_sb")
                masks_sb = consts.tile([128, 4 * SB], BF16, name="masks_sb")

                VT_sb = persist.tile([128, S], BF16, name="VT_sb")

                # --- attention machinery (emitted incrementally) ---
                hilo_tiles = {}
                wd_holder = []

                def attn_head(qb, h, mid=None):
                    # mid: emitted after the first score_exp — V-transpose
                    # groups slot here so their cross-engine latency hides
                    # behind this head's remaining scores
                    nkt = 4 * qb + 4  # causal: k-tiles 0..4qb+3
                    if qb not in hilo_tiles:
                        hilo_tiles[qb] = attnout.tile(
                            [128, NH_LOC, 2, SB], F8, name="hilo", tag="attn")
                    hilo = hilo_tiles[qb]
                    den_ps = psden.tile([128, SB], F32, name="den_ps", tag="den")
                    att_ps = pspv.tile([128, SB], F32, name="att_ps", tag="pv")
                    split = nkt <= 4
                    E_tiles = []

                    def score_exp(kt):
                        d = kt - 4 * qb
                        off = 128 * d if d >= 1 else 0
                        st_ps = ps.tile([128, SB], F32, name="st_ps", tag="ps")
                        nc.tensor.matmul(
                            st_ps[:, off:],
                            lhsT=KT_sb[:, kt * 128:(kt + 1) * 128],
                            rhs=QT_sb[:, h, qb * SB + off:(qb + 1) * SB],
                            start=True,
                            stop=True,
                        )
                        E1 = erot.tile([128, SB], BF16, name="E1", tag="E")
                        nc.scalar.activation(
                            E1[:, off:], st_ps[:, off:],
                            mybir.ActivationFunctionType.Exp,
                            scale=EXP_SCALE,
                        )
                        if d >= 0:
                            nc.vector.tensor_tensor(
                                E1[:, off:],
                                E1[:, off:],
                                masks_sb[:, d * SB + off:(d + 1) * SB],
                                mybir.AluOpType.mult,
                            )
                        return E1

                    def den_pv(kt, E1):
                        d = kt - 4 * qb
                        off = 128 * d if d >= 1 else 0
                        nc.tensor.matmul(
                            den_ps[:, off:],
                            lhsT=ones_sb[:, :],
                            rhs=E1[:, off:],
                            start=(kt == 0),
                            stop=(kt == nkt - 1),
                        )
                        nc.tensor.matmul(
                            att_ps[:, off:],
                            lhsT=V_sb[:, kt, :],
                            rhs=E1[:, off:],
                            start=(kt == 0),
                            stop=(kt == nkt - 1),
                        )

                    if split:
                        for kt in range(nkt):
                            E_tiles.append(score_exp(kt))
                            if kt == 0 and mid is not None:
                                mid()
                        for kt in range(nkt):
                            den_pv(kt, E_tiles[kt])
                    else:
                        # scores run one k-tile ahead of den/pv so the PE
                        # never waits out the exp+mask latency
                        E_prev = None
                        for kt in range(nkt):
                            E1 = score_exp(kt)
                            if kt == 0 and mid is not None:
                                mid()
                            if E_prev is not None:
                                den_pv(kt - 1, E_prev)
                            E_prev = E1
                        den_pv(nkt - 1, E_prev)
                    # normalize + split to fp8 hi/lo (attn scaled x16)
                    rec = small.tile([128, SB], F32, name="rec", tag="rec")
                    nc.vector.reciprocal(rec[:], den_ps[:])
                    abf = abfp.tile([128, SB], BF16, name="abf", tag="abf")
                    nc.vector.tensor_tensor(
                        abf[:], att_ps[:], rec[:], mybir.AluOpType.mult
                    )
                    nc.scalar.copy(hilo[:, h, 0, :], abf[:])
                    nc.vector.tensor_tensor(
                        hilo[:, h, 1, :], abf[:], hilo[:, h, 0, :],
                        mybir.AluOpType.subtract,
                    )
                    nc.sync.dma_start(
                        attn_loc[qb][h * 256:h * 256 + 256, :]
                        .rearrange("(hl p) q -> p hl q", p=128),
                        hilo[:, h, :, :],
                    )

                def attn_gather(qb):
                    if stage >= 4:
                        nc.gpsimd.collective_compute(
                            "AllGather",
                            mybir.AluOpType.bypass,
                            replica_groups=[list(range(N_CORES))],
                            ins=[attn_loc[qb][:]],
                            outs=[attn_gath[qb][:]],
                        )

                def wd_slice(qb):
                    if qb == 0:
                        wd_holder.append(bigw.tile(
                            [128, NDC, 2, FLOC], F8, name="wd_sb", tag="bigw"))
                    wdc = NDC // NSB
                    nc.sync.dma_start(
                        wd_holder[0][:, wdc * qb:wdc * (qb + 1), :, :],
                        wdT[:, wdc * qb:wdc * (qb + 1), :, :])

                def transp_group(g, pool, tag):
                    tp = pool.tile([128, 4, 128], BF16, name="tp", tag=tag)
                    for j in range(4):
                        st = 4 * g + j
                        nc.tensor.transpose(
                            tp[:, j, :], VT_sb[:, st * 128:(st + 1) * 128],
                            ident_sb[:])
                    nc.scalar.copy(V_sb[:, 4 * g:4 * (g + 1), :], tp[:])

                # --- V phase A (cols 0:512, sb0) — 12 Q units run here:
                # block 7's 8 plus block 6's deferred 4 (heads 2,3). Units are
                # emitted before the V-proj matmuls: their inputs (qt pairs)
                # land before vt does, so they fill the F-tail DMA window.
                # b6 units first (earliest-arriving qt, and each head's QTacc
                # chain must run b6 before b7).
                h1_units = [(NQB - 2, 2, 0), (NQB - 2, 2, 1),
                            (NQB - 2, 3, 0), (NQB - 2, 3, 1),
                            (NQB - 1, 0, 0), (NQB - 1, 0, 1),
                            (NQB - 1, 1, 0), (NQB - 1, 1, 1),
                            (NQB - 1, 2, 0), (NQB - 1, 2, 1),
                            (NQB - 1, 3, 0), (NQB - 1, 3, 1)]
                attn_jobs = [(0, 0), (0, 1), (0, 2), (0, 3),
                             (1, 0), (1, 1), (1, 2), (1, 3)]
                emitted = 0

                def pump_attn(n, mid=None):
                    nonlocal emitted
                    for _ in range(n):
                        if emitted >= len(attn_jobs):
                            if mid is not None:
                                mid()
                            return
                        qb, h = attn_jobs[emitted]
                        attn_head(qb, h, mid=mid)
                        mid = None
                        emitted += 1
                        if qb == 0 and h == NH_LOC - 1:
                            attn_gather(0)
                        if qb == 1 and h == NH_LOC - 1:
                            attn_gather(1)

                v_psA = ps.tile([128, SB], F32, name="v_psA", tag="ps")
                ui = 0
                for pr in range(NPR):
                    if pr % BPP == 1 and pr < NPR - BPP:
                        j = pr // BPP + 1
                        nc.sync.dma_start(
                            wv_sb[:, BLK * j:BLK * (j + 1), :, :],
                            wvT[:, BLK * j:BLK * (j + 1), :, :])
                    vt_t = vstream.tile([128, 2, 2, SB], F8,
                                        name="vt_t", tag="vt")
                    nc.sync.dma_start(vt_t[:], vT[:, pr, :, :, 0:SB])
                    if pr == 0:
                        while ui < 4:
                            emit_q_unit(*h1_units[ui])
                            ui += 1
                    elif ui < len(h1_units):
                        emit_q_unit(*h1_units[ui])
                        ui += 1
                    proj_pair([v_psA], wv_sb, vt_t, pr, [0])
                while ui < len(h1_units):
                    emit_q_unit(*h1_units[ui])
                    ui += 1
                nc.scalar.copy(VT_sb[:, 0:SB], v_psA[:])
                nc.sync.dma_start(ident_sb[:], ident[:])
                nc.sync.dma_start(masks_sb[:], masks[:])

                # --- V phase B (cols 512:1024, sb1) with qb0 woven ---
                v_psB = ps.tile([128, SB], F32, name="v_psB", tag="ps")
                for pr in range(NPR):
                    vt_t = vstream.tile([128, 2, 2, SB], F8,
                                        name="vt_t", tag="vt")
                    nc.sync.dma_start(vt_t[:], vT[:, pr, :, :, SB:2 * SB])
                    if stage >= 2 and pr % 4 == 0:
                        pump_attn(1, mid=(
                            (lambda: transp_group(0, pspv, "pv"))
                            if pr == 0 else None))
                    proj_pair([v_psB], wv_sb, vt_t, pr, [0])
                nc.scalar.copy(VT_sb[:, SB:2 * SB], v_psB[:])

                # --- V phase C (cols 1024:2048, sb2+sb3) with qb1 woven ---
                v_psC = [ps.tile([128, SB], F32, name=f"v_psC{i}", tag="ps")
                         for i in range(2)]
                for pr in range(NPR):
                    vt_t = vstream.tile([128, 2, 2, 2 * SB], F8,
                                        name="vt_t", tag="vt")
                    nc.sync.dma_start(vt_t[:], vT[:, pr, :, :, 2 * SB:4 * SB])
                    if stage >= 2 and pr % 4 == 0:
                        pump_attn(1, mid=(
                            (lambda: transp_group(1, psden, "den"))
                            if pr == 0 else None))
                    proj_pair(v_psC, wv_sb, vt_t, pr, range(2))
                for sb in range(2):
                    nc.scalar.copy(VT_sb[:, (2 + sb) * SB:(3 + sb) * SB],
                                   v_psC[sb][:])
                for g in range(2, 4):
                    transp_group(g, ps, "ps")

                # ---- rest of attention (qb1 remainder, qb2, qb3) ----
                if stage >= 2:
                    pump_attn(len(attn_jobs) - emitted)
                    wd_slice(0)
                    wd_slice(1)
                    for qb in range(2, NSB):
                        for h in range(NH_LOC):
                            attn_head(qb, h)
                        attn_gather(qb)
                        wd_slice(qb)

                # ---- output projection per q-block ----
                if stage >= 4:
                    def op_block(qb, dsubs):
                        o_ps = [
                            ps.tile([128, SB], F32, name=f"ops{d2}", tag="ps")
                            if d2 < 2 else
                            (pspv.tile([128, SB], F32, name=f"ops{d2}", tag="pv")
                             if d2 == 2 else
                             psden.tile([128, SB], F32, name=f"ops{d2}", tag="den"))
                            for d2 in dsubs
                        ]
                        for fc2 in range(NDC // 2):
                            at = atin.tile([128, 2, 2, SB], F8, name="at_c",
                                           tag="atin")
                            rowstart = fc2 * 512
                            nc.sync.dma_start(
                                at[:],
                                attn_gath[qb][rowstart:rowstart + 512, :]
                                .rearrange("(j hl p) q -> p j hl q", p=128, hl=2),
                            )
                            for j, dsub in enumerate(dsubs):
                                fcols = slice(dsub * 128, (dsub + 1) * 128)
                                nc.tensor.matmul(
                                    o_ps[j][:],
                                    lhsT=wd_holder[0][:, 2 * fc2:2 * fc2 + 2, 1, fcols],
                                    rhs=at[:, :, 0, :],
                                    start=(fc2 == 0), stop=False, perf_mode=DR,
                                )
                                for j2 in range(2):
                                    nc.tensor.matmul(
                                        o_ps[j][:],
                                        lhsT=wd_holder[0][:, 2 * fc2 + j2, :, fcols],
                                        rhs=at[:, j2, :, :],
                                        start=False,
                                        stop=(fc2 == NDC // 2 - 1 and j2 == 1),
                                        perf_mode=DR,
                                    )
                        # descale into one tile; two DMAs so the first pair's
                        # writeback overlaps the second pair's descale
                        o_all = osb.tile([128, 4, SB], BF16, name="o_all",
                                         tag="osb")
                        for j, dsub in enumerate(dsubs):
                            if j % 2 == 0:
                                nc.vector.tensor_scalar_mul(
                                    o_all[:, dsub, :], o_ps[j][:], OUT_DESCALE)
                            else:
                                nc.scalar.mul(
                                    o_all[:, dsub, :], o_ps[j][:], OUT_DESCALE)
                            if j == 1:
                                nc.sync.dma_start(
                                    outT[0:2 * 128, qb * SB:(qb + 1) * SB]
                                    .rearrange("(j p) q -> p j q", p=128),
                                    o_all[:, 0:2, :],
                                )
                        nc.sync.dma_start(
                            outT[2 * 128:FLOC, qb * SB:(qb + 1) * SB]
                            .rearrange("(j p) q -> p j q", p=128),
                            o_all[:, 2:4, :],
                        )

                    for qb in range(NSB):
                        op_block(qb, [0, 1, 2, 3])

            for rep in range(nrep):
                one_rep(rep)

    nc.compile()
    _legalize_dma_waits(nc)
    nc.codegen_inst_isa_subclasses()
    return nc


_NC_CACHE = None


def _get_nc():
    global _NC_CACHE
    if _NC_CACHE is None:
        _NC_CACHE = _build()
    return _NC_CACHE


def _split8(x, scale):
    """x (f32 [D, S]) -> hi, lo fp8 arrays of the scaled value."""
    f8 = ml_dtypes.float8_e4m3
    xs = x * np.float32(scale)
    hi = xs.astype(f8)
    lo = (xs - hi.astype(np.float32)).astype(f8)
    return hi, lo


def _act_layout(xT_hi, xT_lo):
    """[D, S] fp8 pair -> [128, NPR, 2, 2, S]."""
    h = xT_hi.reshape(NPR, 2, 128, S)
    l = xT_lo.reshape(NPR, 2, 128, S)
    out = np.empty((128, NPR, 2, 2, S), dtype=xT_hi.dtype)
    out[:, :, :, 0, :] = h.transpose(2, 0, 1, 3)
    out[:, :, :, 1, :] = l.transpose(2, 0, 1, 3)
    return np.ascontiguousarray(out)


def _w_layout(wT_hi, wT_lo):
    """[D, F] fp8 pair -> [128, NDC, 2(lo,hi), F]."""
    f = wT_hi.shape[1]
    h = wT_hi.reshape(NDC, 128, f)
    l = wT_lo.reshape(NDC, 128, f)
    out = np.empty((128, NDC, 2, f), dtype=wT_hi.dtype)
    out[:, :, 0, :] = l.transpose(1, 0, 2)
    out[:, :, 1, :] = h.transpose(1, 0, 2)
    return np.ascontiguousarray(out)


def _make_in_maps(q, k, v, Wq, Wk, Wv, Wd):
    bf = ml_dtypes.bfloat16
    scale = np.float32(DK) ** -0.5
    qT = np.ascontiguousarray(q.reshape(S, D).T)
    kT = np.ascontiguousarray(k.reshape(S, D).T)
    vT = np.ascontiguousarray(v.reshape(S, D).T)
    qA = _act_layout(*_split8(qT, S_A))
    kA = _act_layout(*_split8(kT, S_A))
    vA = _act_layout(*_split8(vT, S_A))

    kp = np.arange(128, dtype=np.int32)[:, None]
    qf = np.arange(SB, dtype=np.int32)[None, :]
    masks = np.concatenate(
        [(qf >= kp + 128 * d).astype(np.float32) for d in range(4)], axis=1
    ).astype(bf)
    ident = np.eye(128, dtype=np.float32).astype(bf)

    in_maps = []
    for c in range(N_CORES):
        fs = slice(FLOC * c, FLOC * (c + 1))
        ks = slice(DK * c, DK * (c + 1))
        in_maps.append({
            "qT": qA,
            "kT": kA,
            "vT": vA,
            "wqT": _w_layout(*_split8((Wq[fs, :] * scale).T, S_WQ)),
            "wkT": _w_layout(*_split8(Wk[ks, :].T, S_WKV)),
            "wvT": _w_layout(*_split8(Wv[ks, :].T, S_WKV)),
            "wdT": _w_layout(*_split8(Wd[fs, :].T, S_WD)),
            "masks": masks,
            "ident": ident,
        })
    return in_maps


def _assemble(results):
    outT_full = np.concatenate(
        [r["outT"].astype(np.float32) for r in results], axis=0)  # [4096, 2048]
    return np.ascontiguousarray(outT_full.T).reshape(1, S, D).astype(np.float32)


def kernel(q, k, v, Wq, Wk, Wv, Wd, _trace=False, **_ignored):
    nc = _get_nc()
    in_maps = _make_in_maps(
        np.asarray(q, np.float32), np.asarray(k, np.float32),
        np.asarray(v, np.float32), np.asarray(Wq, np.float32),
        np.asarray(Wk, np.float32), np.asarray(Wv, np.float32),
        np.asarray(Wd, np.float32),
    )
    res = run_bass_kernel_spmd(
        nc, in_maps, core_ids=list(range(N_CORES)), trace=_trace
    )
    out = _assemble(res.results)
    if _trace:
        return out, res
    return out


# revision 53
# speedup vs baseline: 42965.2800x; 1.0014x over previous
"""Trainium2 Bass kernel for GQA MultiHeadAttention (B=1, S=2048, D=4096,
H=32 query heads, HKV=8 kv heads, DK=DV=128), tensor-parallel across heads
on 8 NeuronCores.

Sharding: core c owns query heads 4c..4c+3 and kv head c (GQA group) and
computes its 512 attention features. The transposed attention output is
AllGathered across cores in four per-q-block collectives (hidden behind
later blocks' compute), then each core computes a 512-row slice of the
transposed output projection.

Schedule (PE ~88.5% busy; sim 331.5us vs 383.7us bf16 baseline):
- F: kt/qt chunk-pair streams + K-proj, with Q-proj "units" (one head x
  one sb-pair x one contraction block) woven in as their inputs land.
  Contraction blocks are variable-size pair-lists: a 1-pair block 0
  (units start ~7us in, filling the DMA-bound warmup) and a 1-pair
  block 8 whose short units defer into V phase A.
- V streams in three column phases A(sb0)/B(sb1)/C(sb2+3) so V k-tiles
  complete progressively: deferred Q units fill A; attention qb0 (plus
  the V-transposes, slotted mid-head behind its first score) weaves
  into B; qb1 into C. Each phase's PSUM->SBUF copies go to the Act
  engine to stay off the DVE's QTacc queue.
- Attention: fused score->exp->mask->den/pv per k-tile, scores running
  one k-tile ahead of den/pv to hide the exp+mask latency; diagonal
  tiles skip fully-masked leading columns (exact). Normalized attn is
  split to fp8 hi/lo (Act cast + DVE subtract) with rows (head,hl,p)
  so the out-proj consumer reads one contiguous block per fc pair.
- Out-proj per q-block; the last gather hides behind the first three
  blocks; all 4 dsubs descale into one tile with two output DMAs.

fp8 acceleration: all four projection GEMMs (Q/K/V/out) run as 3-term
hi/lo float8e4 splits on the PE's DoubleRow mode (2 stacked 128-deep
k-tiles per instruction at 0.5 cycles/col = 4x bf16 throughput):
    x @ W ~= x8@W8 (hi*hi, DoubleRow over chunk pairs)
           + [dW8@x8 + W8@dx8] (one DoubleRow per chunk, lo/hi stacked)
Weights and streamed activations are hi/lo-split and interleaved on the
host; the attention output is split on-device (Act cast + DVE subtract)
before the AllGather. Attention internals (scores/exp/mask/PV/den) stay
bf16: fp8 scores or probabilities fail the 2e-2 gate (measured 4.9e-2),
and raw exp() values overflow fp8's dynamic range.

Scales (powers of 2, exact): activations x16; Wq (with DK^-0.5 folded)
x2^13; Wk/Wv/Wd x2^9. The ones-matrix for the denominator is 512 so the
reciprocal folds the descale and attn comes out x16 ready for fp8; exp
descales scores by 2^-30 via the activation scale; output descales by
2^-13 into bf16 (host upcasts to f32).

Self-contained: hardcodes all shapes; inputs are the full unsharded tensors
keyed as in the problem's setup_inputs().
"""

import numpy as np
import ml_dtypes

import concourse.bacc as bacc
import concourse.mybir as mybir
from concourse.tile import TileContext
from concourse.bass_utils import run_bass_kernel_spmd

BF16 = mybir.dt.bfloat16
F8 = mybir.dt.float8e4
F32 = mybir.dt.float32
DR = mybir.MatmulPerfMode.DoubleRow

N_CORES = 8
S = 2048            # sequence length
D = 4096            # model dim
DK = 128            # head dim
NH_LOC = 4          # query heads per core
FLOC = NH_LOC * DK  # per-core attention features (512)
NDC = D // 128      # contraction chunks of 128 over D (32)
NPR = NDC // 2      # chunk pairs (16)
SB = 512            # q/s block width
NSB = S // SB       # 4
NST = S // 128      # 16 seq tiles of 128

S_A = 16.0          # activation scale
S_WQ = 2.0 ** 13    # Wq scale (DK^-0.5 folded first)
S_WKV = 2.0 ** 9
S_WD = 2.0 ** 9
ONES_VAL = 512.0    # den descale so attn psum*rec = 16*attn
EXP_SCALE = 2.0 ** -30
OUT_DESCALE = 2.0 ** -13

_DMA_TYPES = ("InstDMACopy", "InstDMATranspose")


def _legalize_dma_waits(nc):
    """DMA pseudo-instructions encode at most ONE sem wait (the ISA events
    slot). If Tile's sem assignment leaves more on a DMA, walrus rejects it
    ("Too many sync wait commands"). Hoist all but the last wait onto fresh
    nop instructions inserted immediately before the DMA on the same engine —
    the sequencer executes them in order, so semantics are identical."""
    ctr = 0
    for f in nc.m.functions:
        for blk in f.blocks:
            out = []
            changed = False
            for inst in blk.instructions:
                si = inst.sync_info
                if (
                    si is not None
                    and len(si.on_wait) > 1
                    and type(inst).__name__ in _DMA_TYPES
                ):
                    waits = list(si.on_wait)
                    for w in waits[:-1]:
                        nop = mybir.InstNoOp(
                            name=f"I-dmawaitfix-{ctr}", ins=[], outs=[]
                        )
                        ctr += 1
                        nop.engine = inst.engine
                        nop.sync_info = mybir.SyncInfo(on_wait=[w], on_update=[])
                        out.append(nop)
                    inst.sync_info = mybir.SyncInfo(
                        on_wait=[waits[-1]], on_update=list(si.on_update)
                    )
                    changed = True
                out.append(inst)
            if changed:
                blk.instructions = out
    return ctr


def _build(stage=4, nrep=1):
    nc = bacc.Bacc("TRN2", target_bir_lowering=False, num_devices=N_CORES,
                   dynamic_dma_scratch_size=2048)

    # ---- I/O ----
    # activations: [128, NPR, 2(chunk-in-pair), 2(hi,lo), S] fp8
    qT = nc.dram_tensor("qT", [128, NPR, 2, 2, S], F8, kind="ExternalInput")
    kT = nc.dram_tensor("kT", [128, NPR, 2, 2, S], F8, kind="ExternalInput")
    vT = nc.dram_tensor("vT", [128, NPR, 2, 2, S], F8, kind="ExternalInput")
    # weights: [128, NDC, 2(lo,hi), F] fp8 — hl order REVERSED vs
    # activations so one DoubleRow computes w_lo.T@x_hi + w_hi.T@x_lo
    wqT = nc.dram_tensor("wqT", [128, NDC, 2, FLOC], F8, kind="ExternalInput")
    wkT = nc.dram_tensor("wkT", [128, NDC, 2, DK], F8, kind="ExternalInput")
    wvT = nc.dram_tensor("wvT", [128, NDC, 2, DK], F8, kind="ExternalInput")
    wdT = nc.dram_tensor("wdT", [128, NDC, 2, FLOC], F8, kind="ExternalInput")
    masks = nc.dram_tensor("masks", [128, 4 * SB], BF16, kind="ExternalInput")
    ident = nc.dram_tensor("ident", [128, 128], BF16, kind="ExternalInput")
    outT = nc.dram_tensor("outT", [FLOC, S], BF16, kind="ExternalOutput")

    NBLK = 8
    BLK = NDC // NBLK  # 4 chunks (2 pairs) per Q contraction block
    BPP = BLK // 2     # pairs per block (2)

    with TileContext(nc) as tc:
        with (
            tc.tile_pool(name="consts", bufs=1) as consts,
            tc.tile_pool(name="kvw", bufs=1) as kvw,
            tc.tile_pool(name="bigw", bufs=1) as bigw,
            tc.tile_pool(name="persist", bufs=1) as persist,
            tc.tile_pool(name="qstream", bufs=4) as qstream,
            tc.tile_pool(name="kstream", bufs=3) as kstream,
            tc.tile_pool(name="vstream", bufs=6) as vstream,
            tc.tile_pool(name="erot", bufs=6) as erot,
            tc.tile_pool(name="small", bufs=1) as small,
            tc.tile_pool(name="attnout", bufs=2) as attnout,
            tc.tile_pool(name="abf", bufs=2) as abfp,
            tc.tile_pool(name="atin", bufs=6) as atin,
            tc.tile_pool(name="osb", bufs=2) as osb,
            tc.tile_pool(name="ps", bufs=4, space="PSUM") as ps,
            tc.tile_pool(name="pspv", bufs=2, space="PSUM") as pspv,
            tc.tile_pool(name="psden", bufs=2, space="PSUM") as psden,
            tc.tile_pool(name="dram", bufs=1, space="DRAM") as dram,
        ):
            def one_rep(rep):
                # wk first: K-proj's first matmul waits only on wk + kt[0]
                wk_sb = kvw.tile([128, NDC, 2, DK], F8, name="wk_sb", tag="kvw")
                nc.sync.dma_start(wk_sb[:, 0:BLK, :, :], wkT[:, 0:BLK, :, :])
                ones_sb = consts.tile([128, 128], BF16, name="ones_sb")
                nc.vector.memset(ones_sb[:], ONES_VAL)

                # persistent activations
                QT_sb = persist.tile([128, NH_LOC, S], BF16, name="QT_sb")
                KT_sb = persist.tile([128, S], BF16, name="KT_sb")
                V_sb = persist.tile([128, NST, DK], BF16, name="V_sb")
                QTacc = persist.tile([128, NH_LOC, S], BF16, name="QTacc")

                # per-q-block DRAM bounce buffers for the collectives
                # rows: (head, hl, p) so the consumer reads one contiguous
                # 512-row block per fc2 pair
                attn_loc = [
                    dram.tile([2 * FLOC, SB], F8, name=f"attn_loc{qb}", tag=f"al{qb}")
                    for qb in range(NSB)
                ]
                attn_gath = [
                    dram.tile([N_CORES * 2 * FLOC, SB], F8, name=f"attn_gath{qb}",
                              tag=f"ag{qb}", addr_space="Shared")
                    for qb in range(NSB)
                ]

                # ---- front: K-proj and Q-proj interleaved at pair level ----
                q_pairs = {}
                wq_holder = []

                # Q contraction blocks as pair-lists: a 1-pair block 0 (its
                # units start right after qt0 — fills the DMA-bound warmup)
                # and a 1-pair block 8 at the end (short units that defer
                # cheaply into V phase A).
                PAIR_BLOCKS = ([[0]] + [[2 * i + 1, 2 * i + 2]
                                        for i in range(7)] + [[15]])
                NQB = len(PAIR_BLOCKS)  # 9

                def emit_q_unit(b, f, pair):
                    # one head-feature (f) x one sb-pair of contraction block b
                    tpool, ttag = (pspv, "pv") if pair == 0 else (psden, "den")
                    qp = [
                        tpool.tile([128, SB], F32, name=f"qp{j}", tag=ttag)
                        for j in range(2)
                    ]
                    prs = PAIR_BLOCKS[b]
                    for pj, pr in enumerate(prs):
                        dc0 = 2 * pr
                        qt = q_pairs[pr]
                        for j2 in range(2):
                            s2 = 2 * pair + j2
                            cols = slice(s2 * SB, (s2 + 1) * SB)
                            fcols = slice(f * 128, (f + 1) * 128)
                            nc.tensor.matmul(
                                qp[j2][:],
                                lhsT=wq_holder[0][:, dc0:dc0 + 2, 1, fcols],
                                rhs=qt[:, :, 0, cols],
                                start=(pj == 0), stop=False, perf_mode=DR,
                            )
                            for j in range(2):
                                nc.tensor.matmul(
                                    qp[j2][:],
                                    lhsT=wq_holder[0][:, dc0 + j, :, fcols],
                                    rhs=qt[:, j, :, cols],
                                    start=False,
                                    stop=(pj == len(prs) - 1 and j == 1),
                                    perf_mode=DR,
                                )
                    for j in range(2):
                        s2 = 2 * pair + j
                        dst_acc = QTacc[:, f, s2 * SB:(s2 + 1) * SB]
                        if b == 0:
                            nc.vector.tensor_copy(dst_acc, qp[j][:])
                        elif b < NQB - 1:
                            nc.vector.tensor_tensor(
                                dst_acc, dst_acc, qp[j][:], mybir.AluOpType.add
                            )
                        else:
                            nc.vector.tensor_tensor(
                                QT_sb[:, f, s2 * SB:(s2 + 1) * SB],
                                dst_acc, qp[j][:], mybir.AluOpType.add,
                            )

                def proj_pair(ps_tiles, w_sb, x_t, pr, sbs):
                    # 3-term hi/lo DoubleRow for one chunk pair
                    dc0 = 2 * pr
                    for sb in sbs:
                        cols = slice(sb * SB, (sb + 1) * SB)
                        nc.tensor.matmul(
                            ps_tiles[sb][:],
                            lhsT=w_sb[:, dc0:dc0 + 2, 1, :],
                            rhs=x_t[:, :, 0, cols],
                            start=(pr == 0), stop=False, perf_mode=DR,
                        )
                        for j in range(2):
                            nc.tensor.matmul(
                                ps_tiles[sb][:],
                                lhsT=w_sb[:, dc0 + j, :, :],
                                rhs=x_t[:, j, :, cols],
                                start=False,
                                stop=(pr == NPR - 1 and j == 1),
                                perf_mode=DR,
                            )

                k_ps = [ps.tile([128, SB], F32, name=f"kps{i}", tag="ps")
                        for i in range(NSB)]
                # F emits blocks 0..7 minus block 7's heads 2,3 (those 4 and
                # all of block 8 defer into V phase A). Availability: block b
                # is runnable once its last qt pair (pr = 2b for b>=1, pr 0
                # for b0) and wq pair-slices have landed.
                f_units = [(b, f, pair) for b in range(NQB - 1)
                           for f in range(NH_LOC) for pair in range(2)][:-4]
                fui = 0
                for pr in range(NPR):
                    kt_t = kstream.tile([128, 2, 2, S], F8, name="kt_t",
                                        tag="kt")
                    nc.sync.dma_start(kt_t[:], kT[:, pr, :, :, :])
                    qt_t = qstream.tile([128, 2, 2, S], F8, name="qt_t",
                                        tag="qt")
                    nc.sync.dma_start(qt_t[:], qT[:, pr, :, :, :])
                    q_pairs[pr] = qt_t
                    if pr == 0:
                        wq_sb = bigw.tile([128, NDC, 2, FLOC], F8,
                                          name="wq_sb", tag="bigw")
                        wq_holder.append(wq_sb)
                    if pr % BPP == 0 and pr > 0:
                        dc0 = 2 * pr
                        nc.sync.dma_start(
                            wk_sb[:, dc0:dc0 + BLK, :, :],
                            wkT[:, dc0:dc0 + BLK, :, :])
                    # wq pair-slice (small, so block 0's units unblock early)
                    nc.sync.dma_start(
                        wq_sb[:, 2 * pr:2 * pr + 2, :, :],
                        wqT[:, 2 * pr:2 * pr + 2, :, :])
                    # units BEFORE this pair's K-proj: by the time the PE
                    # reaches them their qt pairs have arrived, while K-proj
                    # waits on this pair's kt DMA — in-order PE. Exception:
                    # at pr0, kt0 lands before qt0, so K-proj goes first.
                    avail = 8 * (1 + pr // 2)
                    target = min(len(f_units), avail, 2 + 4 * pr)
                    if pr == 0:
                        proj_pair(k_ps, wk_sb, kt_t, pr, range(NSB))
                    while fui < target:
                        emit_q_unit(*f_units[fui])
                        fui += 1
                    if pr > 0:
                        proj_pair(k_ps, wk_sb, kt_t, pr, range(NSB))
                for sb in range(NSB):
                    nc.vector.tensor_copy(KT_sb[:, sb * SB:(sb + 1) * SB], k_ps[sb][:])

                # ---- V projection in column halves so k-tiles 0..7 are
                # ---- ready at half-stream: Q's last block weaves into half 1
                # ---- (PE-idle DMA windows), attention qb0/qb1 into half 2.
                wv_sb = kvw.tile([128, NDC, 2, DK], F8, name="wv_sb", tag="kvw")
                nc.sync.dma_start(wv_sb[:, 0:BLK, :, :], wvT[:, 0:BLK, :, :])
                ident_sb = consts.tile([128, 128], BF16, name="ident_sb")
                masks_sb = consts.tile([128, 4 * SB], BF16, name="masks_sb")

                VT_sb = persist.tile([128, S], BF16, name="VT_sb")

                # --- attention machinery (emitted incrementally) ---
                hilo_tiles = {}
                wd_holder = []

                def attn_head(qb, h, mid=None):
                    # mid: emitted after the first score_exp — V-transpose
                    # groups slot here so their cross-engine latency hides
                    # behind this head's remaining scores
                    nkt = 4 * qb + 4  # causal: k-tiles 0..4qb+3
                    if qb not in hilo_tiles:
                        hilo_tiles[qb] = attnout.tile(
                            [128, NH_LOC, 2, SB], F8, name="hilo", tag="attn")
                    hilo = hilo_tiles[qb]
                    den_ps = psden.tile([128, SB], F32, name="den_ps", tag="den")
                    att_ps = pspv.tile([128, SB], F32, name="att_ps", tag="pv")
                    split = nkt <= 4
                    E_tiles = []

                    def score_exp(kt):
                        d = kt - 4 * qb
                        off = 128 * d if d >= 1 else 0
                        st_ps = ps.tile([128, SB], F32, name="st_ps", tag="ps")
                        nc.tensor.matmul(
                            st_ps[:, off:],
                            lhsT=KT_sb[:, kt * 128:(kt + 1) * 128],
                            rhs=QT_sb[:, h, qb * SB + off:(qb + 1) * SB],
                            start=True,
                            stop=True,
                        )
                        E1 = erot.tile([128, SB], BF16, name="E1", tag="E")
                        nc.scalar.activation(
                            E1[:, off:], st_ps[:, off:],
                            mybir.ActivationFunctionType.Exp,
                            scale=EXP_SCALE,
                        )
                        if d >= 0:
                            nc.vector.tensor_tensor(
                                E1[:, off:],
                                E1[:, off:],
                                masks_sb[:, d * SB + off:(d + 1) * SB],
                                mybir.AluOpType.mult,
                            )
                        return E1

                    def den_pv(kt, E1):
                        d = kt - 4 * qb
                        off = 128 * d if d >= 1 else 0
                        nc.tensor.matmul(
                            den_ps[:, off:],
                            lhsT=ones_sb[:, :],
                            rhs=E1[:, off:],
                            start=(kt == 0),
                            stop=(kt == nkt - 1),
                        )
                        nc.tensor.matmul(
                            att_ps[:, off:],
                            lhsT=V_sb[:, kt, :],
                            rhs=E1[:, off:],
                            start=(kt == 0),
                            stop=(kt == nkt - 1),
                        )

                    if split:
                        for kt in range(nkt):
                            E_tiles.append(score_exp(kt))
                            if kt == 0 and mid is not None:
                                mid()
                        for kt in range(nkt):
                            den_pv(kt, E_tiles[kt])
                    else:
                        # scores run one k-tile ahead of den/pv so the PE
                        # never waits out the exp+mask latency
                        E_prev = None
                        for kt in range(nkt):
                            E1 = score_exp(kt)
                            if kt == 0 and mid is not None:
                                mid()
                            if E_prev is not None:
                                den_pv(kt - 1, E_prev)
                            E_prev = E1
                        den_pv(nkt - 1, E_prev)
                    # normalize + split to fp8 hi/lo (attn scaled x16)
                    rec = small.tile([128, SB], F32, name="rec", tag="rec")
                    nc.vector.reciprocal(rec[:], den_ps[:])
                    abf = abfp.tile([128, SB], BF16, name="abf", tag="abf")
                    nc.vector.tensor_tensor(
                        abf[:], att_ps[:], rec[:], mybir.AluOpType.mult
                    )
                    nc.scalar.copy(hilo[:, h, 0, :], abf[:])
                    nc.vector.tensor_tensor(
                        hilo[:, h, 1, :], abf[:], hilo[:, h, 0, :],
                        mybir.AluOpType.subtract,
                    )
                    nc.sync.dma_start(
                        attn_loc[qb][h * 256:h * 256 + 256, :]
                        .rearrange("(hl p) q -> p hl q", p=128),
                        hilo[:, h, :, :],
                    )

                def attn_gather(qb):
                    if stage >= 4:
                        nc.gpsimd.collective_compute(
                            "AllGather",
                            mybir.AluOpType.bypass,
                            replica_groups=[list(range(N_CORES))],
                            ins=[attn_loc[qb][:]],
                            outs=[attn_gath[qb][:]],
                        )

                def wd_slice(qb):
                    if qb == 0:
                        wd_holder.append(bigw.tile(
                            [128, NDC, 2, FLOC], F8, name="wd_sb", tag="bigw"))
                    wdc = NDC // NSB
                    nc.sync.dma_start(
                        wd_holder[0][:, wdc * qb:wdc * (qb + 1), :, :],
                        wdT[:, wdc * qb:wdc * (qb + 1), :, :])

                def transp_group(g, pool, tag):
                    tp = pool.tile([128, 4, 128], BF16, name="tp", tag=tag)
                    for j in range(4):
                        st = 4 * g + j
                        nc.tensor.transpose(
                            tp[:, j, :], VT_sb[:, st * 128:(st + 1) * 128],
                            ident_sb[:])
                    nc.scalar.copy(V_sb[:, 4 * g:4 * (g + 1), :], tp[:])

                # --- V phase A (cols 0:512, sb0) — 12 Q units run here:
                # block 7's 8 plus block 6's deferred 4 (heads 2,3). Units are
                # emitted before the V-proj matmuls: their inputs (qt pairs)
                # land before vt does, so they fill the F-tail DMA window.
                # b6 units first (earliest-arriving qt, and each head's QTacc
                # chain must run b6 before b7).
                h1_units = [(NQB - 2, 2, 0), (NQB - 2, 2, 1),
                            (NQB - 2, 3, 0), (NQB - 2, 3, 1),
                            (NQB - 1, 0, 0), (NQB - 1, 0, 1),
                            (NQB - 1, 1, 0), (NQB - 1, 1, 1),
                            (NQB - 1, 2, 0), (NQB - 1, 2, 1),
                            (NQB - 1, 3, 0), (NQB - 1, 3, 1)]
                attn_jobs = [(0, 0), (0, 1), (0, 2), (0, 3),
                             (1, 0), (1, 1), (1, 2), (1, 3)]
                emitted = 0

                def pump_attn(n, mid=None):
                    nonlocal emitted
                    for _ in range(n):
                        if emitted >= len(attn_jobs):
                            if mid is not None:
                                mid()
                            return
                        qb, h = attn_jobs[emitted]
                        attn_head(qb, h, mid=mid)
                        mid = None
                        emitted += 1
                        if qb == 0 and h == NH_LOC - 1:
                            attn_gather(0)
                        if qb == 1 and h == NH_LOC - 1:
                            attn_gather(1)

                v_psA = ps.tile([128, SB], F32, name="v_psA", tag="ps")
                ui = 0
                for pr in range(NPR):
                    if pr % BPP == 1 and pr < NPR - BPP:
                        j = pr // BPP + 1
                        nc.sync.dma_start(
                            wv_sb[:, BLK * j:BLK * (j + 1), :, :],
                            wvT[:, BLK * j:BLK * (j + 1), :, :])
                    vt_t = vstream.tile([128, 2, 2, SB], F8,
                                        name="vt_t", tag="vt")
                    nc.sync.dma_start(vt_t[:], vT[:, pr, :, :, 0:SB])
                    if pr == 11:
                        nc.sync.dma_start(ident_sb[:], ident[:])
                        nc.sync.dma_start(masks_sb[:], masks[:])
                    if pr == 0:
                        while ui < 4:
                            emit_q_unit(*h1_units[ui])
                            ui += 1
                    elif ui < len(h1_units):
                        emit_q_unit(*h1_units[ui])
                        ui += 1
                    proj_pair([v_psA], wv_sb, vt_t, pr, [0])
                while ui < len(h1_units):
                    emit_q_unit(*h1_units[ui])
                    ui += 1
                nc.scalar.copy(VT_sb[:, 0:SB], v_psA[:])

                # --- V phase B (cols 512:1024, sb1) with qb0 woven ---
                v_psB = ps.tile([128, SB], F32, name="v_psB", tag="ps")
                for pr in range(NPR):
                    vt_t = vstream.tile([128, 2, 2, SB], F8,
                                        name="vt_t", tag="vt")
                    nc.sync.dma_start(vt_t[:], vT[:, pr, :, :, SB:2 * SB])
                    if stage >= 2 and pr % 4 == 0:
                        pump_attn(1, mid=(
                            (lambda: transp_group(0, pspv, "pv"))
                            if pr == 0 else None))
                    proj_pair([v_psB], wv_sb, vt_t, pr, [0])
                nc.scalar.copy(VT_sb[:, SB:2 * SB], v_psB[:])

                # --- V phase C (cols 1024:2048, sb2+sb3) with qb1 woven ---
                v_psC = [ps.tile([128, SB], F32, name=f"v_psC{i}", tag="ps")
                         for i in range(2)]
                for pr in range(NPR):
                    vt_t = vstream.tile([128, 2, 2, 2 * SB], F8,
                                        name="vt_t", tag="vt")
                    nc.sync.dma_start(vt_t[:], vT[:, pr, :, :, 2 * SB:4 * SB])
                    if stage >= 2 and pr % 4 == 0:
                        pump_attn(1, mid=(
                            (lambda: transp_group(1, psden, "den"))
                            if pr == 0 else None))
                    proj_pair(v_psC, wv_sb, vt_t, pr, range(2))
                for sb in range(2):
                    nc.scalar.copy(VT_sb[:, (2 + sb) * SB:(3 + sb) * SB],
                                   v_psC[sb][:])
                for g in range(2, 4):
                    transp_group(g, ps, "ps")

                # ---- rest of attention (qb1 remainder, qb2, qb3) ----
                if stage >= 2:
                    pump_attn(len(attn_jobs) - emitted)
                    wd_slice(0)
                    wd_slice(1)
                    for qb in range(2, NSB):
                        for h in range(NH_LOC):
                            attn_head(qb, h)
                        attn_gather(qb)
                        wd_slice(qb)

                # ---- output projection per q-block ----
                if stage >= 4:
                    def op_block(qb, dsubs):
                        o_ps = [
                            ps.tile([128, SB], F32, name=f"ops{d2}", tag="ps")
                            if d2 < 2 else
                            (pspv.tile([128, SB], F32, name=f"ops{d2}", tag="pv")
                             if d2 == 2 else
                             psden.tile([128, SB], F32, name=f"ops{d2}", tag="den"))
                            for d2 in dsubs
                        ]
                        for fc2 in range(NDC // 2):
                            at = atin.tile([128, 2, 2, SB], F8, name="at_c",
                                           tag="atin")
                            rowstart = fc2 * 512
                            nc.sync.dma_start(
                                at[:],
                                attn_gath[qb][rowstart:rowstart + 512, :]
                                .rearrange("(j hl p) q -> p j hl q", p=128, hl=2),
                            )
                            for j, dsub in enumerate(dsubs):
                                fcols = slice(dsub * 128, (dsub + 1) * 128)
                                nc.tensor.matmul(
                                    o_ps[j][:],
                                    lhsT=wd_holder[0][:, 2 * fc2:2 * fc2 + 2, 1, fcols],
                                    rhs=at[:, :, 0, :],
                                    start=(fc2 == 0), stop=False, perf_mode=DR,
                                )
                                for j2 in range(2):
                                    nc.tensor.matmul(
                                        o_ps[j][:],
                                        lhsT=wd_holder[0][:, 2 * fc2 + j2, :, fcols],
                                        rhs=at[:, j2, :, :],
                                        start=False,
                                        stop=(fc2 == NDC // 2 - 1 and j2 == 1),
                                        perf_mode=DR,
                                    )
                        # descale into one tile; two DMAs so the first pair's
                        # writeback overlaps the second pair's descale
                        o_all = osb.tile([128, 4, SB], BF16, name="o_all",
                                         tag="osb")
                        for j, dsub in enumerate(dsubs):
                            if j % 2 == 0:
                                nc.vector.tensor_scalar_mul(
                                    o_all[:, dsub, :], o_ps[j][:], OUT_DESCALE)
                            else:
                                nc.scalar.mul(
                                    o_all[:, dsub, :], o_ps[j][:], OUT_DESCALE)
                            if j == 1:
                                nc.sync.dma_start(
                                    outT[0:2 * 128, qb * SB:(qb + 1) * SB]
                                    .rearrange("(j p) q -> p j q", p=128),
                                    o_all[:, 0:2, :],
                                )
                        nc.sync.dma_start(
                            outT[2 * 128:FLOC, qb * SB:(qb + 1) * SB]
                            .rearrange("(j p) q -> p j q", p=128),
                            o_all[:, 2:4, :],
                        )

                    for qb in range(NSB):
                        op_block(qb, [0, 1, 2, 3])

            for rep in range(nrep):
                one_rep(rep)

    nc.compile()
    _legalize_dma_waits(nc)
    nc.codegen_inst_isa_subclasses()
    return nc


_NC_CACHE = None


def _get_nc():
    global _NC_CACHE
    if _NC_CACHE is None:
        _NC_CACHE = _build()
    return _NC_CACHE


def _split8(x, scale):
    """x (f32 [D, S]) -> hi, lo fp8 arrays of the scaled value."""
    f8 = ml_dtypes.float8_e4m3
    xs = x * np.float32(scale)
    hi = xs.astype(f8)
    lo = (xs - hi.astype(np.float32)).astype(f8)
    return hi, lo


def _act_layout(xT_hi, xT_lo):
    """[D, S] fp8 pair -> [128, NPR, 2, 2, S]."""
    h = xT_hi.reshape(NPR, 2, 128, S)
    l = xT_lo.reshape(NPR, 2, 128, S)
    out = np.empty((128, NPR, 2, 2, S), dtype=xT_hi.dtype)
    out[:, :, :, 0, :] = h.transpose(2, 0, 1, 3)
    out[:, :, :, 1, :] = l.transpose(2, 0, 1, 3)
    return np.ascontiguousarray(out)


def _w_layout(wT_hi, wT_lo):
    """[D, F] fp8 pair -> [128, NDC, 2(lo,hi), F]."""
    f = wT_hi.shape[1]
    h = wT_hi.reshape(NDC, 128, f)
    l = wT_lo.reshape(NDC, 128, f)
    out = np.empty((128, NDC, 2, f), dtype=wT_hi.dtype)
    out[:, :, 0, :] = l.transpose(1, 0, 2)
    out[:, :, 1, :] = h.transpose(1, 0, 2)
    return np.ascontiguousarray(out)


def _make_in_maps(q, k, v, Wq, Wk, Wv, Wd):
    bf = ml_dtypes.bfloat16
    scale = np.float32(DK) ** -0.5
    qT = np.ascontiguousarray(q.reshape(S, D).T)
    kT = np.ascontiguousarray(k.reshape(S, D).T)
    vT = np.ascontiguousarray(v.reshape(S, D).T)
    qA = _act_layout(*_split8(qT, S_A))
    kA = _act_layout(*_split8(kT, S_A))
    vA = _act_layout(*_split8(vT, S_A))

    kp = np.arange(128, dtype=np.int32)[:, None]
    qf = np.arange(SB, dtype=np.int32)[None, :]
    masks = np.concatenate(
        [(qf >= kp + 128 * d).astype(np.float32) for d in range(4)], axis=1
    ).astype(bf)
    ident = np.eye(128, dtype=np.float32).astype(bf)

    in_maps = []
    for c in range(N_CORES):
        fs = slice(FLOC * c, FLOC * (c + 1))
        ks = slice(DK * c, DK * (c + 1))
        in_maps.append({
            "qT": qA,
            "kT": kA,
            "vT": vA,
            "wqT": _w_layout(*_split8((Wq[fs, :] * scale).T, S_WQ)),
            "wkT": _w_layout(*_split8(Wk[ks, :].T, S_WKV)),
            "wvT": _w_layout(*_split8(Wv[ks, :].T, S_WKV)),
            "wdT": _w_layout(*_split8(Wd[fs, :].T, S_WD)),
            "masks": masks,
            "ident": ident,
        })
    return in_maps


def _assemble(results):
    outT_full = np.concatenate(
        [r["outT"].astype(np.float32) for r in results], axis=0)  # [4096, 2048]
    return np.ascontiguousarray(outT_full.T).reshape(1, S, D).astype(np.float32)


def kernel(q, k, v, Wq, Wk, Wv, Wd, _trace=False, **_ignored):
    nc = _get_nc()
    in_maps = _make_in_maps(
        np.asarray(q, np.float32), np.asarray(k, np.float32),
        np.asarray(v, np.float32), np.asarray(Wq, np.float32),
        np.asarray(Wk, np.float32), np.asarray(Wv, np.float32),
        np.asarray(Wd, np.float32),
    )
    res = run_bass_kernel_spmd(
        nc, in_maps, core_ids=list(range(N_CORES)), trace=_trace
    )
    out = _assemble(res.results)
    if _trace:
        return out, res
    return out


# revision 59
# speedup vs baseline: 42972.9391x; 1.0002x over previous
"""Trainium2 Bass kernel for GQA MultiHeadAttention (B=1, S=2048, D=4096,
H=32 query heads, HKV=8 kv heads, DK=DV=128), tensor-parallel across heads
on 8 NeuronCores.

Sharding: core c owns query heads 4c..4c+3 and kv head c (GQA group) and
computes its 512 attention features. The transposed attention output is
AllGathered across cores in four per-q-block collectives (hidden behind
later blocks' compute), then each core computes a 512-row slice of the
transposed output projection.

Schedule (PE ~88.5% busy; sim 331.5us vs 383.7us bf16 baseline):
- F: kt/qt chunk-pair streams + K-proj, with Q-proj "units" (one head x
  one sb-pair x one contraction block) woven in as their inputs land.
  Contraction blocks are variable-size pair-lists: a 1-pair block 0
  (units start ~7us in, filling the DMA-bound warmup) and a 1-pair
  block 8 whose short units defer into V phase A.
- V streams in three column phases A(sb0)/B(sb1)/C(sb2+3) so V k-tiles
  complete progressively: deferred Q units fill A; attention qb0 (plus
  the V-transposes, slotted mid-head behind its first score) weaves
  into B; qb1 into C. Each phase's PSUM->SBUF copies go to the Act
  engine to stay off the DVE's QTacc queue.
- Attention: fused score->exp->mask->den/pv per k-tile, scores running
  one k-tile ahead of den/pv to hide the exp+mask latency; diagonal
  tiles skip fully-masked leading columns (exact). Normalized attn is
  split to fp8 hi/lo (Act cast + DVE subtract) with rows (head,hl,p)
  so the out-proj consumer reads one contiguous block per fc pair.
- Out-proj per q-block; the last gather hides behind the first three
  blocks; all 4 dsubs descale into one tile with two output DMAs.

fp8 acceleration: all four projection GEMMs (Q/K/V/out) run as 3-term
hi/lo float8e4 splits on the PE's DoubleRow mode (2 stacked 128-deep
k-tiles per instruction at 0.5 cycles/col = 4x bf16 throughput):
    x @ W ~= x8@W8 (hi*hi, DoubleRow over chunk pairs)
           + [dW8@x8 + W8@dx8] (one DoubleRow per chunk, lo/hi stacked)
Weights and streamed activations are hi/lo-split and interleaved on the
host; the attention output is split on-device (Act cast + DVE subtract)
before the AllGather. Attention internals (scores/exp/mask/PV/den) stay
bf16: fp8 scores or probabilities fail the 2e-2 gate (measured 4.9e-2),
and raw exp() values overflow fp8's dynamic range.

Scales (powers of 2, exact): activations x16; Wq (with DK^-0.5 folded)
x2^13; Wk/Wv/Wd x2^9. The ones-matrix for the denominator is 512 so the
reciprocal folds the descale and attn comes out x16 ready for fp8; exp
descales scores by 2^-30 via the activation scale; output descales by
2^-13 into bf16 (host upcasts to f32).

Self-contained: hardcodes all shapes; inputs are the full unsharded tensors
keyed as in the problem's setup_inputs().
"""

import numpy as np
import ml_dtypes

import concourse.bacc as bacc
import concourse.mybir as mybir
from concourse.tile import TileContext
from concourse.bass_utils import run_bass_kernel_spmd

BF16 = mybir.dt.bfloat16
F8 = mybir.dt.float8e4
F32 = mybir.dt.float32
DR = mybir.MatmulPerfMode.DoubleRow

N_CORES = 8
S = 2048            # sequence length
D = 4096            # model dim
DK = 128            # head dim
NH_LOC = 4          # query heads per core
FLOC = NH_LOC * DK  # per-core attention features (512)
NDC = D // 128      # contraction chunks of 128 over D (32)
NPR = NDC // 2      # chunk pairs (16)
SB = 512            # q/s block width
NSB = S // SB       # 4
NST = S // 128      # 16 seq tiles of 128

S_A = 16.0          # activation scale
S_WQ = 2.0 ** 13    # Wq scale (DK^-0.5 folded first)
S_WKV = 2.0 ** 9
S_WD = 2.0 ** 9
ONES_VAL = 512.0    # den descale so attn psum*rec = 16*attn
EXP_SCALE = 2.0 ** -30
OUT_DESCALE = 2.0 ** -13

_DMA_TYPES = ("InstDMACopy", "InstDMATranspose")


def _legalize_dma_waits(nc):
    """DMA pseudo-instructions encode at most ONE sem wait (the ISA events
    slot). If Tile's sem assignment leaves more on a DMA, walrus rejects it
    ("Too many sync wait commands"). Hoist all but the last wait onto fresh
    nop instructions inserted immediately before the DMA on the same engine —
    the sequencer executes them in order, so semantics are identical."""
    ctr = 0
    for f in nc.m.functions:
        for blk in f.blocks:
            out = []
            changed = False
            for inst in blk.instructions:
                si = inst.sync_info
                if (
                    si is not None
                    and len(si.on_wait) > 1
                    and type(inst).__name__ in _DMA_TYPES
                ):
                    waits = list(si.on_wait)
                    for w in waits[:-1]:
                        nop = mybir.InstNoOp(
                            name=f"I-dmawaitfix-{ctr}", ins=[], outs=[]
                        )
                        ctr += 1
                        nop.engine = inst.engine
                        nop.sync_info = mybir.SyncInfo(on_wait=[w], on_update=[])
                        out.append(nop)
                    inst.sync_info = mybir.SyncInfo(
                        on_wait=[waits[-1]], on_update=list(si.on_update)
                    )
                    changed = True
                out.append(inst)
            if changed:
                blk.instructions = out
    return ctr


def _build(stage=4, nrep=1):
    nc = bacc.Bacc("TRN2", target_bir_lowering=False, num_devices=N_CORES,
                   dynamic_dma_scratch_size=2048)

    # ---- I/O ----
    # activations: [128, NPR, 2(chunk-in-pair), 2(hi,lo), S] fp8
    qT = nc.dram_tensor("qT", [128, NPR, 2, 2, S], F8, kind="ExternalInput")
    kT = nc.dram_tensor("kT", [128, NPR, 2, 2, S], F8, kind="ExternalInput")
    vT = nc.dram_tensor("vT", [128, NPR, 2, 2, S], F8, kind="ExternalInput")
    # weights: [128, NDC, 2(lo,hi), F] fp8 — hl order REVERSED vs
    # activations so one DoubleRow computes w_lo.T@x_hi + w_hi.T@x_lo
    wqT = nc.dram_tensor("wqT", [128, NDC, 2, FLOC], F8, kind="ExternalInput")
    wkT = nc.dram_tensor("wkT", [128, NDC, 2, DK], F8, kind="ExternalInput")
    wvT = nc.dram_tensor("wvT", [128, NDC, 2, DK], F8, kind="ExternalInput")
    wdT = nc.dram_tensor("wdT", [128, NDC, 2, FLOC], F8, kind="ExternalInput")
    masks = nc.dram_tensor("masks", [128, 4 * SB], BF16, kind="ExternalInput")
    ident = nc.dram_tensor("ident", [128, 128], BF16, kind="ExternalInput")
    outT = nc.dram_tensor("outT", [FLOC, S], BF16, kind="ExternalOutput")

    NBLK = 8
    BLK = NDC // NBLK  # 4 chunks (2 pairs) per Q contraction block
    BPP = BLK // 2     # pairs per block (2)

    with TileContext(nc) as tc:
        with (
            tc.tile_pool(name="consts", bufs=1) as consts,
            tc.tile_pool(name="kvw", bufs=1) as kvw,
            tc.tile_pool(name="bigw", bufs=1) as bigw,
            tc.tile_pool(name="persist", bufs=1) as persist,
            tc.tile_pool(name="qstream", bufs=4) as qstream,
            tc.tile_pool(name="kstream", bufs=3) as kstream,
            tc.tile_pool(name="vstream", bufs=6) as vstream,
            tc.tile_pool(name="erot", bufs=6) as erot,
            tc.tile_pool(name="small", bufs=1) as small,
            tc.tile_pool(name="attnout", bufs=2) as attnout,
            tc.tile_pool(name="abf", bufs=2) as abfp,
            tc.tile_pool(name="atin", bufs=6) as atin,
            tc.tile_pool(name="osb", bufs=2) as osb,
            tc.tile_pool(name="ps", bufs=4, space="PSUM") as ps,
            tc.tile_pool(name="pspv", bufs=2, space="PSUM") as pspv,
            tc.tile_pool(name="psden", bufs=2, space="PSUM") as psden,
            tc.tile_pool(name="dram", bufs=1, space="DRAM") as dram,
        ):
            def one_rep(rep):
                # wk first: K-proj's first matmul waits only on wk + kt[0]
                wk_sb = kvw.tile([128, NDC, 2, DK], F8, name="wk_sb", tag="kvw")
                nc.sync.dma_start(wk_sb[:, 0:BLK, :, :], wkT[:, 0:BLK, :, :])
                ones_sb = consts.tile([128, 128], BF16, name="ones_sb")
                nc.vector.memset(ones_sb[:], ONES_VAL)

                # persistent activations
                QT_sb = persist.tile([128, NH_LOC, S], BF16, name="QT_sb")
                KT_sb = persist.tile([128, S], BF16, name="KT_sb")
                V_sb = persist.tile([128, NST, DK], BF16, name="V_sb")
                QTacc = persist.tile([128, NH_LOC, S], BF16, name="QTacc")

                # per-q-block DRAM bounce buffers for the collectives
                # rows: (head, hl, p) so the consumer reads one contiguous
                # 512-row block per fc2 pair
                attn_loc = [
                    dram.tile([2 * FLOC, SB], F8, name=f"attn_loc{qb}", tag=f"al{qb}")
                    for qb in range(NSB)
                ]
                attn_gath = [
                    dram.tile([N_CORES * 2 * FLOC, SB], F8, name=f"attn_gath{qb}",
                              tag=f"ag{qb}", addr_space="Shared")
                    for qb in range(NSB)
                ]

                # ---- front: K-proj and Q-proj interleaved at pair level ----
                q_pairs = {}
                wq_holder = []

                # Q contraction blocks as pair-lists: a 1-pair block 0 (its
                # units start right after qt0 — fills the DMA-bound warmup)
                # and a 1-pair block 8 at the end (short units that defer
                # cheaply into V phase A).
                PAIR_BLOCKS = ([[0]] + [[2 * i + 1, 2 * i + 2]
                                        for i in range(7)] + [[15]])
                NQB = len(PAIR_BLOCKS)  # 9

                def emit_q_unit(b, f, pair):
                    # one head-feature (f) x one sb-pair of contraction block b
                    tpool, ttag = (pspv, "pv") if pair == 0 else (psden, "den")
                    qp = [
                        tpool.tile([128, SB], F32, name=f"qp{j}", tag=ttag)
                        for j in range(2)
                    ]
                    prs = PAIR_BLOCKS[b]
                    for pj, pr in enumerate(prs):
                        dc0 = 2 * pr
                        qt = q_pairs[pr]
                        for j2 in range(2):
                            s2 = 2 * pair + j2
                            cols = slice(s2 * SB, (s2 + 1) * SB)
                            fcols = slice(f * 128, (f + 1) * 128)
                            nc.tensor.matmul(
                                qp[j2][:],
                                lhsT=wq_holder[0][:, dc0:dc0 + 2, 1, fcols],
                                rhs=qt[:, :, 0, cols],
                                start=(pj == 0), stop=False, perf_mode=DR,
                            )
                            for j in range(2):
                                nc.tensor.matmul(
                                    qp[j2][:],
                                    lhsT=wq_holder[0][:, dc0 + j, :, fcols],
                                    rhs=qt[:, j, :, cols],
                                    start=False,
                                    stop=(pj == len(prs) - 1 and j == 1),
                                    perf_mode=DR,
                                )
                    for j in range(2):
                        s2 = 2 * pair + j
                        dst_acc = QTacc[:, f, s2 * SB:(s2 + 1) * SB]
                        if b == 0:
                            nc.vector.tensor_copy(dst_acc, qp[j][:])
                        elif b < NQB - 1:
                            nc.vector.tensor_tensor(
                                dst_acc, dst_acc, qp[j][:], mybir.AluOpType.add
                            )
                        else:
                            nc.vector.tensor_tensor(
                                QT_sb[:, f, s2 * SB:(s2 + 1) * SB],
                                dst_acc, qp[j][:], mybir.AluOpType.add,
                            )

                def proj_pair(ps_tiles, w_sb, x_t, pr, sbs):
                    # 3-term hi/lo DoubleRow for one chunk pair
                    dc0 = 2 * pr
                    for sb in sbs:
                        cols = slice(sb * SB, (sb + 1) * SB)
                        nc.tensor.matmul(
                            ps_tiles[sb][:],
                            lhsT=w_sb[:, dc0:dc0 + 2, 1, :],
                            rhs=x_t[:, :, 0, cols],
                            start=(pr == 0), stop=False, perf_mode=DR,
                        )
                        for j in range(2):
                            nc.tensor.matmul(
                                ps_tiles[sb][:],
                                lhsT=w_sb[:, dc0 + j, :, :],
                                rhs=x_t[:, j, :, cols],
                                start=False,
                                stop=(pr == NPR - 1 and j == 1),
                                perf_mode=DR,
                            )

                k_ps = [ps.tile([128, SB], F32, name=f"kps{i}", tag="ps")
                        for i in range(NSB)]
                # F emits blocks 0..7 minus block 7's heads 2,3 (those 4 and
                # all of block 8 defer into V phase A). Availability: block b
                # is runnable once its last qt pair (pr = 2b for b>=1, pr 0
                # for b0) and wq pair-slices have landed.
                f_units = [(b, f, pair) for b in range(NQB - 1)
                           for f in range(NH_LOC) for pair in range(2)][:-4]
                fui = 0
                for pr in range(NPR):
                    kt_t = kstream.tile([128, 2, 2, S], F8, name="kt_t",
                                        tag="kt")
                    nc.sync.dma_start(kt_t[:], kT[:, pr, :, :, :])
                    qt_t = qstream.tile([128, 2, 2, S], F8, name="qt_t",
                                        tag="qt")
                    nc.sync.dma_start(qt_t[:], qT[:, pr, :, :, :])
                    q_pairs[pr] = qt_t
                    if pr == 0:
                        wq_sb = bigw.tile([128, NDC, 2, FLOC], F8,
                                          name="wq_sb", tag="bigw")
                        wq_holder.append(wq_sb)
                    if pr % BPP == 0 and pr > 0:
                        dc0 = 2 * pr
                        nc.sync.dma_start(
                            wk_sb[:, dc0:dc0 + BLK, :, :],
                            wkT[:, dc0:dc0 + BLK, :, :])
                    # wq pair-slice (small, so block 0's units unblock early)
                    nc.sync.dma_start(
                        wq_sb[:, 2 * pr:2 * pr + 2, :, :],
                        wqT[:, 2 * pr:2 * pr + 2, :, :])
                    # units BEFORE this pair's K-proj: by the time the PE
                    # reaches them their qt pairs have arrived, while K-proj
                    # waits on this pair's kt DMA — in-order PE. Exception:
                    # at pr0, kt0 lands before qt0, so K-proj goes first.
                    avail = 8 * (1 + pr // 2)
                    target = min(len(f_units), avail, 2 + 4 * pr)
                    if pr == 0:
                        proj_pair(k_ps, wk_sb, kt_t, pr, range(NSB))
                    while fui < target:
                        emit_q_unit(*f_units[fui])
                        fui += 1
                    if pr > 0:
                        proj_pair(k_ps, wk_sb, kt_t, pr, range(NSB))
                for sb in range(NSB):
                    # Act, not DVE: keeps the F-end DVE queue clear for the
                    # V-A units' QTacc adds (their psum-reuse WAR gate)
                    nc.scalar.copy(KT_sb[:, sb * SB:(sb + 1) * SB], k_ps[sb][:])

                # ---- V projection in column halves so k-tiles 0..7 are
                # ---- ready at half-stream: Q's last block weaves into half 1
                # ---- (PE-idle DMA windows), attention qb0/qb1 into half 2.
                wv_sb = kvw.tile([128, NDC, 2, DK], F8, name="wv_sb", tag="kvw")
                nc.sync.dma_start(wv_sb[:, 0:BLK, :, :], wvT[:, 0:BLK, :, :])
                ident_sb = consts.tile([128, 128], BF16, name="ident_sb")
                masks_sb = consts.tile([128, 4 * SB], BF16, name="masks_sb")

                VT_sb = persist.tile([128, S], BF16, name="VT_sb")

                # --- attention machinery (emitted incrementally) ---
                hilo_tiles = {}
                wd_holder = []

                def attn_head(qb, h, mid=None):
                    # mid: emitted after the first score_exp — V-transpose
                    # groups slot here so their cross-engine latency hides
                    # behind this head's remaining scores
                    nkt = 4 * qb + 4  # causal: k-tiles 0..4qb+3
                    if qb not in hilo_tiles:
                        hilo_tiles[qb] = attnout.tile(
                            [128, NH_LOC, 2, SB], F8, name="hilo", tag="attn")
                    hilo = hilo_tiles[qb]
                    den_ps = psden.tile([128, SB], F32, name="den_ps", tag="den")
                    att_ps = pspv.tile([128, SB], F32, name="att_ps", tag="pv")
                    split = nkt <= 4
                    E_tiles = []

                    def score_exp(kt):
                        d = kt - 4 * qb
                        off = 128 * d if d >= 1 else 0
                        st_ps = ps.tile([128, SB], F32, name="st_ps", tag="ps")
                        nc.tensor.matmul(
                            st_ps[:, off:],
                            lhsT=KT_sb[:, kt * 128:(kt + 1) * 128],
                            rhs=QT_sb[:, h, qb * SB + off:(qb + 1) * SB],
                            start=True,
                            stop=True,
                        )
                        E1 = erot.tile([128, SB], BF16, name="E1", tag="E")
                        nc.scalar.activation(
                            E1[:, off:], st_ps[:, off:],
                            mybir.ActivationFunctionType.Exp,
                            scale=EXP_SCALE,
                        )
                        if d >= 0:
                            nc.vector.tensor_tensor(
                                E1[:, off:],
                                E1[:, off:],
                                masks_sb[:, d * SB + off:(d + 1) * SB],
                                mybir.AluOpType.mult,
                            )
                        return E1

                    def den_pv(kt, E1):
                        d = kt - 4 * qb
                        off = 128 * d if d >= 1 else 0
                        nc.tensor.matmul(
                            den_ps[:, off:],
                            lhsT=ones_sb[:, :],
                            rhs=E1[:, off:],
                            start=(kt == 0),
                            stop=(kt == nkt - 1),
                        )
                        nc.tensor.matmul(
                            att_ps[:, off:],
                            lhsT=V_sb[:, kt, :],
                            rhs=E1[:, off:],
                            start=(kt == 0),
                            stop=(kt == nkt - 1),
                        )

                    if split:
                        for kt in range(nkt):
                            E_tiles.append(score_exp(kt))
                            if kt == 0 and mid is not None:
                                mid()
                        for kt in range(nkt):
                            den_pv(kt, E_tiles[kt])
                    else:
                        # scores run one k-tile ahead of den/pv so the PE
                        # never waits out the exp+mask latency
                        E_prev = None
                        for kt in range(nkt):
                            E1 = score_exp(kt)
                            if kt == 0 and mid is not None:
                                mid()
                            if E_prev is not None:
                                den_pv(kt - 1, E_prev)
                            E_prev = E1
                        den_pv(nkt - 1, E_prev)
                    # normalize + split to fp8 hi/lo (attn scaled x16)
                    rec = small.tile([128, SB], F32, name="rec", tag="rec")
                    nc.vector.reciprocal(rec[:], den_ps[:])
                    abf = abfp.tile([128, SB], BF16, name="abf", tag="abf")
                    nc.vector.tensor_tensor(
                        abf[:], att_ps[:], rec[:], mybir.AluOpType.mult
                    )
                    nc.scalar.copy(hilo[:, h, 0, :], abf[:])
                    nc.vector.tensor_tensor(
                        hilo[:, h, 1, :], abf[:], hilo[:, h, 0, :],
                        mybir.AluOpType.subtract,
                    )
                    nc.sync.dma_start(
                        attn_loc[qb][h * 256:h * 256 + 256, :]
                        .rearrange("(hl p) q -> p hl q", p=128),
                        hilo[:, h, :, :],
                    )

                def attn_gather(qb):
                    if stage >= 4:
                        nc.gpsimd.collective_compute(
                            "AllGather",
                            mybir.AluOpType.bypass,
                            replica_groups=[list(range(N_CORES))],
                            ins=[attn_loc[qb][:]],
                            outs=[attn_gath[qb][:]],
                        )

                def wd_slice(qb):
                    if qb == 0:
                        wd_holder.append(bigw.tile(
                            [128, NDC, 2, FLOC], F8, name="wd_sb", tag="bigw"))
                    wdc = NDC // NSB
                    nc.sync.dma_start(
                        wd_holder[0][:, wdc * qb:wdc * (qb + 1), :, :],
                        wdT[:, wdc * qb:wdc * (qb + 1), :, :])

                def transp_group(g, pool, tag):
                    tp = pool.tile([128, 4, 128], BF16, name="tp", tag=tag)
                    for j in range(4):
                        st = 4 * g + j
                        nc.tensor.transpose(
                            tp[:, j, :], VT_sb[:, st * 128:(st + 1) * 128],
                            ident_sb[:])
                    nc.scalar.copy(V_sb[:, 4 * g:4 * (g + 1), :], tp[:])

                # --- V phase A (cols 0:512, sb0) — 12 Q units run here:
                # block 7's 8 plus block 6's deferred 4 (heads 2,3). Units are
                # emitted before the V-proj matmuls: their inputs (qt pairs)
                # land before vt does, so they fill the F-tail DMA window.
                # b6 units first (earliest-arriving qt, and each head's QTacc
                # chain must run b6 before b7).
                h1_units = [(NQB - 2, 2, 0), (NQB - 2, 2, 1),
                            (NQB - 2, 3, 0), (NQB - 2, 3, 1),
                            (NQB - 1, 0, 0), (NQB - 1, 0, 1),
                            (NQB - 1, 1, 0), (NQB - 1, 1, 1),
                            (NQB - 1, 2, 0), (NQB - 1, 2, 1),
                            (NQB - 1, 3, 0), (NQB - 1, 3, 1)]
                attn_jobs = [(0, 0), (0, 1), (0, 2), (0, 3),
                             (1, 0), (1, 1), (1, 2), (1, 3)]
                emitted = 0

                def pump_attn(n, mid=None):
                    nonlocal emitted
                    for _ in range(n):
                        if emitted >= len(attn_jobs):
                            if mid is not None:
                                mid()
                            return
                        qb, h = attn_jobs[emitted]
                        attn_head(qb, h, mid=mid)
                        mid = None
                        emitted += 1
                        if qb == 0 and h == NH_LOC - 1:
                            attn_gather(0)
                        if qb == 1 and h == NH_LOC - 1:
                            attn_gather(1)

                v_psA = ps.tile([128, SB], F32, name="v_psA", tag="ps")
                ui = 0
                for pr in range(NPR):
                    if pr % BPP == 1 and pr < NPR - BPP:
                        j = pr // BPP + 1
                        nc.sync.dma_start(
                            wv_sb[:, BLK * j:BLK * (j + 1), :, :],
                            wvT[:, BLK * j:BLK * (j + 1), :, :])
                    vt_t = vstream.tile([128, 2, 2, SB], F8,
                                        name="vt_t", tag="vt")
                    nc.sync.dma_start(vt_t[:], vT[:, pr, :, :, 0:SB])
                    if pr == 11:
                        nc.sync.dma_start(ident_sb[:], ident[:])
                        nc.sync.dma_start(masks_sb[:], masks[:])
                    if pr == 0:
                        while ui < 4:
                            emit_q_unit(*h1_units[ui])
                            ui += 1
                    elif ui < len(h1_units):
                        emit_q_unit(*h1_units[ui])
                        ui += 1
                    proj_pair([v_psA], wv_sb, vt_t, pr, [0])
                while ui < len(h1_units):
                    emit_q_unit(*h1_units[ui])
                    ui += 1
                nc.scalar.copy(VT_sb[:, 0:SB], v_psA[:])

                # --- V phase B (cols 512:1024, sb1) with qb0 woven ---
                v_psB = ps.tile([128, SB], F32, name="v_psB", tag="ps")
                for pr in range(NPR):
                    vt_t = vstream.tile([128, 2, 2, SB], F8,
                                        name="vt_t", tag="vt")
                    nc.sync.dma_start(vt_t[:], vT[:, pr, :, :, SB:2 * SB])
                    if stage >= 2 and pr % 4 == 0:
                        pump_attn(1, mid=(
                            (lambda: transp_group(0, pspv, "pv"))
                            if pr == 0 else None))
                    proj_pair([v_psB], wv_sb, vt_t, pr, [0])
                nc.scalar.copy(VT_sb[:, SB:2 * SB], v_psB[:])

                # --- V phase C (cols 1024:2048, sb2+sb3) with qb1 woven ---
                v_psC = [ps.tile([128, SB], F32, name=f"v_psC{i}", tag="ps")
                         for i in range(2)]
                for pr in range(NPR):
                    vt_t = vstream.tile([128, 2, 2, 2 * SB], F8,
                                        name="vt_t", tag="vt")
                    nc.sync.dma_start(vt_t[:], vT[:, pr, :, :, 2 * SB:4 * SB])
                    if stage >= 2 and pr % 4 == 0:
                        pump_attn(1, mid=(
                            (lambda: transp_group(1, psden, "den"))
                            if pr == 0 else None))
                    proj_pair(v_psC, wv_sb, vt_t, pr, range(2))
                for sb in range(2):
                    nc.scalar.copy(VT_sb[:, (2 + sb) * SB:(3 + sb) * SB],
                                   v_psC[sb][:])
                for g in range(2, 4):
                    transp_group(g, ps, "ps")

                # ---- rest of attention (qb1 remainder, qb2, qb3) ----
                if stage >= 2:
                    pump_attn(len(attn_jobs) - emitted)
                    wd_slice(0)
                    wd_slice(1)
                    for qb in range(2, NSB):
                        for h in range(NH_LOC):
                            attn_head(qb, h)
                        attn_gather(qb)
                        wd_slice(qb)

                # ---- output projection per q-block ----
                if stage >= 4:
                    def op_block(qb, dsubs):
                        o_ps = [
                            ps.tile([128, SB], F32, name=f"ops{d2}", tag="ps")
                            if d2 < 2 else
                            (pspv.tile([128, SB], F32, name=f"ops{d2}", tag="pv")
                             if d2 == 2 else
                             psden.tile([128, SB], F32, name=f"ops{d2}", tag="den"))
                            for d2 in dsubs
                        ]
                        for fc2 in range(NDC // 2):
                            at = atin.tile([128, 2, 2, SB], F8, name="at_c",
                                           tag="atin")
                            rowstart = fc2 * 512
                            nc.sync.dma_start(
                                at[:],
                                attn_gath[qb][rowstart:rowstart + 512, :]
                                .rearrange("(j hl p) q -> p j hl q", p=128, hl=2),
                            )
                            for j, dsub in enumerate(dsubs):
                                fcols = slice(dsub * 128, (dsub + 1) * 128)
                                nc.tensor.matmul(
                                    o_ps[j][:],
                                    lhsT=wd_holder[0][:, 2 * fc2:2 * fc2 + 2, 1, fcols],
                                    rhs=at[:, :, 0, :],
                                    start=(fc2 == 0), stop=False, perf_mode=DR,
                                )
                                for j2 in range(2):
                                    nc.tensor.matmul(
                                        o_ps[j][:],
                                        lhsT=wd_holder[0][:, 2 * fc2 + j2, :, fcols],
                                        rhs=at[:, j2, :, :],
                                        start=False,
                                        stop=(fc2 == NDC // 2 - 1 and j2 == 1),
                                        perf_mode=DR,
                                    )
                        # descale into one tile; two DMAs so the first pair's
                        # writeback overlaps the second pair's descale
                        o_all = osb.tile([128, 4, SB], BF16, name="o_all",
                                         tag="osb")
                        for j, dsub in enumerate(dsubs):
                            if j % 2 == 0:
                                nc.vector.tensor_scalar_mul(
                                    o_all[:, dsub, :], o_ps[j][:], OUT_DESCALE)
                            else:
                                nc.scalar.mul(
                                    o_all[:, dsub, :], o_ps[j][:], OUT_DESCALE)
                            if j == 1:
                                nc.sync.dma_start(
                                    outT[0:2 * 128, qb * SB:(qb + 1) * SB]
                                    .rearrange("(j p) q -> p j q", p=128),
                                    o_all[:, 0:2, :],
                                )
                        nc.sync.dma_start(
                            outT[2 * 128:FLOC, qb * SB:(qb + 1) * SB]
                            .rearrange("(j p) q -> p j q", p=128),
                            o_all[:, 2:4, :],
                        )

                    for qb in range(NSB):
                        op_block(qb, [0, 1, 2, 3])

            for rep in range(nrep):
                one_rep(rep)

    nc.compile()
    _legalize_dma_waits(nc)
    nc.codegen_inst_isa_subclasses()
    return nc


_NC_CACHE = None


def _get_nc():
    global _NC_CACHE
    if _NC_CACHE is None:
        _NC_CACHE = _build()
    return _NC_CACHE


def _split8(x, scale):
    """x (f32 [D, S]) -> hi, lo fp8 arrays of the scaled value."""
    f8 = ml_dtypes.float8_e4m3
    xs = x * np.float32(scale)
    hi = xs.astype(f8)
    lo = (xs - hi.astype(np.float32)).astype(f8)
    return hi, lo


def _act_layout(xT_hi, xT_lo):
    """[D, S] fp8 pair -> [128, NPR, 2, 2, S]."""
    h = xT_hi.reshape(NPR, 2, 128, S)
    l = xT_lo.reshape(NPR, 2, 128, S)
    out = np.empty((128, NPR, 2, 2, S), dtype=xT_hi.dtype)
    out[:, :, :, 0, :] = h.transpose(2, 0, 1, 3)
    out[:, :, :, 1, :] = l.transpose(2, 0, 1, 3)
    return np.ascontiguousarray(out)


def _w_layout(wT_hi, wT_lo):
    """[D, F] fp8 pair -> [128, NDC, 2(lo,hi), F]."""
    f = wT_hi.shape[1]
    h = wT_hi.reshape(NDC, 128, f)
    l = wT_lo.reshape(NDC, 128, f)
    out = np.empty((128, NDC, 2, f), dtype=wT_hi.dtype)
    out[:, :, 0, :] = l.transpose(1, 0, 2)
    out[:, :, 1, :] = h.transpose(1, 0, 2)
    return np.ascontiguousarray(out)


def _make_in_maps(q, k, v, Wq, Wk, Wv, Wd):
    bf = ml_dtypes.bfloat16
    scale = np.float32(DK) ** -0.5
    qT = np.ascontiguousarray(q.reshape(S, D).T)
    kT = np.ascontiguousarray(k.reshape(S, D).T)
    vT = np.ascontiguousarray(v.reshape(S, D).T)
    qA = _act_layout(*_split8(qT, S_A))
    kA = _act_layout(*_split8(kT, S_A))
    vA = _act_layout(*_split8(vT, S_A))

    kp = np.arange(128, dtype=np.int32)[:, None]
    qf = np.arange(SB, dtype=np.int32)[None, :]
    masks = np.concatenate(
        [(qf >= kp + 128 * d).astype(np.float32) for d in range(4)], axis=1
    ).astype(bf)
    ident = np.eye(128, dtype=np.float32).astype(bf)

    in_maps = []
    for c in range(N_CORES):
        fs = slice(FLOC * c, FLOC * (c + 1))
        ks = slice(DK * c, DK * (c + 1))
        in_maps.append({
            "qT": qA,
            "kT": kA,
            "vT": vA,
            "wqT": _w_layout(*_split8((Wq[fs, :] * scale).T, S_WQ)),
            "wkT": _w_layout(*_split8(Wk[ks, :].T, S_WKV)),
            "wvT": _w_layout(*_split8(Wv[ks, :].T, S_WKV)),
            "wdT": _w_layout(*_split8(Wd[fs, :].T, S_WD)),
            "masks": masks,
            "ident": ident,
        })
    return in_maps


def _assemble(results):
    outT_full = np.concatenate(
        [r["outT"].astype(np.float32) for r in results], axis=0)  # [4096, 2048]
    return np.ascontiguousarray(outT_full.T).reshape(1, S, D).astype(np.float32)


def kernel(q, k, v, Wq, Wk, Wv, Wd, _trace=False, **_ignored):
    nc = _get_nc()
    in_maps = _make_in_maps(
        np.asarray(q, np.float32), np.asarray(k, np.float32),
        np.asarray(v, np.float32), np.asarray(Wq, np.float32),
        np.asarray(Wk, np.float32), np.asarray(Wv, np.float32),
        np.asarray(Wd, np.float32),
    )
    res = run_bass_kernel_spmd(
        nc, in_maps, core_ids=list(range(N_CORES)), trace=_trace
    )
    out = _assemble(res.results)
    if _trace:
        return out, res
    return out


# revision 61
# speedup vs baseline: 43052.5475x; 1.0019x over previous
"""Trainium2 Bass kernel for GQA MultiHeadAttention (B=1, S=2048, D=4096,
H=32 query heads, HKV=8 kv heads, DK=DV=128), tensor-parallel across heads
on 8 NeuronCores.

Sharding: core c owns query heads 4c..4c+3 and kv head c (GQA group) and
computes its 512 attention features. The transposed attention output is
AllGathered across cores in four per-q-block collectives (hidden behind
later blocks' compute), then each core computes a 512-row slice of the
transposed output projection.

Schedule (PE ~88.6% busy; sim 331.0us vs 383.7us bf16 baseline):
- F: kt/qt chunk-pair streams + K-proj, with Q-proj "units" (one head x
  one sb-pair x one contraction block) woven in as their inputs land.
  Contraction blocks are variable-size pair-lists: a 1-pair block 0
  (units start ~7us in, filling the DMA-bound warmup) and a 1-pair
  block 8 whose short units defer into V phase A.
- V streams in three column phases A(sb0)/B(sb1)/C(sb2+3) so V k-tiles
  complete progressively: deferred Q units fill A; attention qb0 (plus
  the V-transposes, slotted mid-head behind its first score) weaves
  into B; qb1 into C. Each phase's PSUM->SBUF copies go to the Act
  engine to stay off the DVE's QTacc queue.
- Attention: fused score->exp->mask->den/pv per k-tile, scores running
  one k-tile ahead of den/pv to hide the exp+mask latency; diagonal
  tiles skip fully-masked leading columns (exact). Normalized attn is
  split to fp8 hi/lo (Act cast + DVE subtract) with rows (head,hl,p)
  so the out-proj consumer reads one contiguous block per fc pair.
- Out-proj per q-block; the last gather hides behind the first three
  blocks; all 4 dsubs descale into one tile with two output DMAs.

fp8 acceleration: all four projection GEMMs (Q/K/V/out) run as 3-term
hi/lo float8e4 splits on the PE's DoubleRow mode (2 stacked 128-deep
k-tiles per instruction at 0.5 cycles/col = 4x bf16 throughput):
    x @ W ~= x8@W8 (hi*hi, DoubleRow over chunk pairs)
           + [dW8@x8 + W8@dx8] (one DoubleRow per chunk, lo/hi stacked)
Weights and streamed activations are hi/lo-split and interleaved on the
host; the attention output is split on-device (Act cast + DVE subtract)
before the AllGather. Attention internals (scores/exp/mask/PV/den) stay
bf16: fp8 scores or probabilities fail the 2e-2 gate (measured 4.9e-2),
and raw exp() values overflow fp8's dynamic range.

Scales (powers of 2, exact): activations x16; Wq (with DK^-0.5 folded)
x2^13; Wk/Wv/Wd x2^9. The ones-matrix for the denominator is 512 so the
reciprocal folds the descale and attn comes out x16 ready for fp8; exp
descales scores by 2^-30 via the activation scale; output descales by
2^-13 into bf16 (host upcasts to f32).

Self-contained: hardcodes all shapes; inputs are the full unsharded tensors
keyed as in the problem's setup_inputs().
"""

import numpy as np
import ml_dtypes

import concourse.bacc as bacc
import concourse.mybir as mybir
from concourse.tile import TileContext
from concourse.bass_utils import run_bass_kernel_spmd

BF16 = mybir.dt.bfloat16
F8 = mybir.dt.float8e4
F32 = mybir.dt.float32
DR = mybir.MatmulPerfMode.DoubleRow

N_CORES = 8
S = 2048            # sequence length
D = 4096            # model dim
DK = 128            # head dim
NH_LOC = 4          # query heads per core
FLOC = NH_LOC * DK  # per-core attention features (512)
NDC = D // 128      # contraction chunks of 128 over D (32)
NPR = NDC // 2      # chunk pairs (16)
SB = 512            # q/s block width
NSB = S // SB       # 4
NST = S // 128      # 16 seq tiles of 128

S_A = 16.0          # activation scale
S_WQ = 2.0 ** 13    # Wq scale (DK^-0.5 folded first)
S_WKV = 2.0 ** 9
S_WD = 2.0 ** 9
ONES_VAL = 512.0    # den descale so attn psum*rec = 16*attn
EXP_SCALE = 2.0 ** -30
OUT_DESCALE = 2.0 ** -13

_DMA_TYPES = ("InstDMACopy", "InstDMATranspose")


def _legalize_dma_waits(nc):
    """DMA pseudo-instructions encode at most ONE sem wait (the ISA events
    slot). If Tile's sem assignment leaves more on a DMA, walrus rejects it
    ("Too many sync wait commands"). Hoist all but the last wait onto fresh
    nop instructions inserted immediately before the DMA on the same engine —
    the sequencer executes them in order, so semantics are identical."""
    ctr = 0
    for f in nc.m.functions:
        for blk in f.blocks:
            out = []
            changed = False
            for inst in blk.instructions:
                si = inst.sync_info
                if (
                    si is not None
                    and len(si.on_wait) > 1
                    and type(inst).__name__ in _DMA_TYPES
                ):
                    waits = list(si.on_wait)
                    for w in waits[:-1]:
                        nop = mybir.InstNoOp(
                            name=f"I-dmawaitfix-{ctr}", ins=[], outs=[]
                        )
                        ctr += 1
                        nop.engine = inst.engine
                        nop.sync_info = mybir.SyncInfo(on_wait=[w], on_update=[])
                        out.append(nop)
                    inst.sync_info = mybir.SyncInfo(
                        on_wait=[waits[-1]], on_update=list(si.on_update)
                    )
                    changed = True
                out.append(inst)
            if changed:
                blk.instructions = out
    return ctr


def _build(stage=4, nrep=1):
    nc = bacc.Bacc("TRN2", target_bir_lowering=False, num_devices=N_CORES,
                   dynamic_dma_scratch_size=2048)

    # ---- I/O ----
    # activations: [128, NPR, 2(chunk-in-pair), 2(hi,lo), S] fp8
    qT = nc.dram_tensor("qT", [128, NPR, 2, 2, S], F8, kind="ExternalInput")
    kT = nc.dram_tensor("kT", [128, NPR, 2, 2, S], F8, kind="ExternalInput")
    vT = nc.dram_tensor("vT", [128, NPR, 2, 2, S], F8, kind="ExternalInput")
    # weights: [128, NDC, 2(lo,hi), F] fp8 — hl order REVERSED vs
    # activations so one DoubleRow computes w_lo.T@x_hi + w_hi.T@x_lo
    wqT = nc.dram_tensor("wqT", [128, NDC, 2, FLOC], F8, kind="ExternalInput")
    wkT = nc.dram_tensor("wkT", [128, NDC, 2, DK], F8, kind="ExternalInput")
    wvT = nc.dram_tensor("wvT", [128, NDC, 2, DK], F8, kind="ExternalInput")
    wdT = nc.dram_tensor("wdT", [128, NDC, 2, FLOC], F8, kind="ExternalInput")
    masks = nc.dram_tensor("masks", [128, 4 * SB], BF16, kind="ExternalInput")
    ident = nc.dram_tensor("ident", [128, 128], BF16, kind="ExternalInput")
    outT = nc.dram_tensor("outT", [FLOC, S], BF16, kind="ExternalOutput")

    NBLK = 8
    BLK = NDC // NBLK  # 4 chunks (2 pairs) per Q contraction block
    BPP = BLK // 2     # pairs per block (2)

    with TileContext(nc) as tc:
        with (
            tc.tile_pool(name="consts", bufs=1) as consts,
            tc.tile_pool(name="kvw", bufs=1) as kvw,
            tc.tile_pool(name="bigw", bufs=1) as bigw,
            tc.tile_pool(name="persist", bufs=1) as persist,
            tc.tile_pool(name="qstream", bufs=4) as qstream,
            tc.tile_pool(name="kstream", bufs=3) as kstream,
            tc.tile_pool(name="vstream", bufs=6) as vstream,
            tc.tile_pool(name="erot", bufs=6) as erot,
            tc.tile_pool(name="small", bufs=1) as small,
            tc.tile_pool(name="attnout", bufs=2) as attnout,
            tc.tile_pool(name="abf", bufs=2) as abfp,
            tc.tile_pool(name="atin", bufs=6) as atin,
            tc.tile_pool(name="osb", bufs=2) as osb,
            tc.tile_pool(name="ps", bufs=4, space="PSUM") as ps,
            tc.tile_pool(name="pspv", bufs=2, space="PSUM") as pspv,
            tc.tile_pool(name="psden", bufs=2, space="PSUM") as psden,
            tc.tile_pool(name="dram", bufs=1, space="DRAM") as dram,
        ):
            def one_rep(rep):
                # wk first: K-proj's first matmul waits only on wk + kt[0]
                wk_sb = kvw.tile([128, NDC, 2, DK], F8, name="wk_sb", tag="kvw")
                nc.sync.dma_start(wk_sb[:, 0:BLK, :, :], wkT[:, 0:BLK, :, :])
                ones_sb = consts.tile([128, 128], BF16, name="ones_sb")
                nc.vector.memset(ones_sb[:], ONES_VAL)

                # persistent activations
                QT_sb = persist.tile([128, NH_LOC, S], BF16, name="QT_sb")
                KT_sb = persist.tile([128, S], BF16, name="KT_sb")
                V_sb = persist.tile([128, NST, DK], BF16, name="V_sb")
                QTacc = persist.tile([128, NH_LOC, S], BF16, name="QTacc")

                # per-q-block DRAM bounce buffers for the collectives
                # rows: (head, hl, p) so the consumer reads one contiguous
                # 512-row block per fc2 pair
                attn_loc = [
                    dram.tile([2 * FLOC, SB], F8, name=f"attn_loc{qb}", tag=f"al{qb}")
                    for qb in range(NSB)
                ]
                attn_gath = [
                    dram.tile([N_CORES * 2 * FLOC, SB], F8, name=f"attn_gath{qb}",
                              tag=f"ag{qb}", addr_space="Shared")
                    for qb in range(NSB)
                ]

                # ---- front: K-proj and Q-proj interleaved at pair level ----
                q_pairs = {}
                wq_holder = []

                # Q contraction blocks as pair-lists: a 1-pair block 0 (its
                # units start right after qt0 — fills the DMA-bound warmup)
                # and a 1-pair block 8 at the end (short units that defer
                # cheaply into V phase A).
                PAIR_BLOCKS = ([[0]] + [[2 * i + 1, 2 * i + 2]
                                        for i in range(7)] + [[15]])
                NQB = len(PAIR_BLOCKS)  # 9

                def emit_q_unit(b, f, pair):
                    # one head-feature (f) x one sb-pair of contraction block b
                    tpool, ttag = (pspv, "pv") if pair == 0 else (psden, "den")
                    qp = [
                        tpool.tile([128, SB], F32, name=f"qp{j}", tag=ttag)
                        for j in range(2)
                    ]
                    prs = PAIR_BLOCKS[b]
                    for pj, pr in enumerate(prs):
                        dc0 = 2 * pr
                        qt = q_pairs[pr]
                        for j2 in range(2):
                            s2 = 2 * pair + j2
                            cols = slice(s2 * SB, (s2 + 1) * SB)
                            fcols = slice(f * 128, (f + 1) * 128)
                            nc.tensor.matmul(
                                qp[j2][:],
                                lhsT=wq_holder[0][:, dc0:dc0 + 2, 1, fcols],
                                rhs=qt[:, :, 0, cols],
                                start=(pj == 0), stop=False, perf_mode=DR,
                            )
                            for j in range(2):
                                nc.tensor.matmul(
                                    qp[j2][:],
                                    lhsT=wq_holder[0][:, dc0 + j, :, fcols],
                                    rhs=qt[:, j, :, cols],
                                    start=False,
                                    stop=(pj == len(prs) - 1 and j == 1),
                                    perf_mode=DR,
                                )
                    for j in range(2):
                        s2 = 2 * pair + j
                        dst_acc = QTacc[:, f, s2 * SB:(s2 + 1) * SB]
                        if b == 0:
                            nc.vector.tensor_copy(dst_acc, qp[j][:])
                        elif b < NQB - 1:
                            nc.vector.tensor_tensor(
                                dst_acc, dst_acc, qp[j][:], mybir.AluOpType.add
                            )
                        else:
                            nc.vector.tensor_tensor(
                                QT_sb[:, f, s2 * SB:(s2 + 1) * SB],
                                dst_acc, qp[j][:], mybir.AluOpType.add,
                            )

                def proj_pair(ps_tiles, w_sb, x_t, pr, sbs):
                    # 3-term hi/lo DoubleRow for one chunk pair
                    dc0 = 2 * pr
                    for sb in sbs:
                        cols = slice(sb * SB, (sb + 1) * SB)
                        nc.tensor.matmul(
                            ps_tiles[sb][:],
                            lhsT=w_sb[:, dc0:dc0 + 2, 1, :],
                            rhs=x_t[:, :, 0, cols],
                            start=(pr == 0), stop=False, perf_mode=DR,
                        )
                        for j in range(2):
                            nc.tensor.matmul(
                                ps_tiles[sb][:],
                                lhsT=w_sb[:, dc0 + j, :, :],
                                rhs=x_t[:, j, :, cols],
                                start=False,
                                stop=(pr == NPR - 1 and j == 1),
                                perf_mode=DR,
                            )

                k_ps = [ps.tile([128, SB], F32, name=f"kps{i}", tag="ps")
                        for i in range(NSB)]
                # F emits blocks 0..7 minus block 7's heads 2,3 (those 4 and
                # all of block 8 defer into V phase A). Availability: block b
                # is runnable once its last qt pair (pr = 2b for b>=1, pr 0
                # for b0) and wq pair-slices have landed.
                f_units = [(b, f, pair) for b in range(NQB - 1)
                           for f in range(NH_LOC) for pair in range(2)][:-4]
                fui = 0
                for pr in range(NPR):
                    kt_t = kstream.tile([128, 2, 2, S], F8, name="kt_t",
                                        tag="kt")
                    nc.sync.dma_start(kt_t[:], kT[:, pr, :, :, :])
                    qt_t = qstream.tile([128, 2, 2, S], F8, name="qt_t",
                                        tag="qt")
                    nc.sync.dma_start(qt_t[:], qT[:, pr, :, :, :])
                    q_pairs[pr] = qt_t
                    if pr == 0:
                        wq_sb = bigw.tile([128, NDC, 2, FLOC], F8,
                                          name="wq_sb", tag="bigw")
                        wq_holder.append(wq_sb)
                    if pr % BPP == 0 and pr > 0:
                        dc0 = 2 * pr
                        nc.sync.dma_start(
                            wk_sb[:, dc0:dc0 + BLK, :, :],
                            wkT[:, dc0:dc0 + BLK, :, :])
                    # wq pair-slice (small, so block 0's units unblock early)
                    nc.sync.dma_start(
                        wq_sb[:, 2 * pr:2 * pr + 2, :, :],
                        wqT[:, 2 * pr:2 * pr + 2, :, :])
                    # units BEFORE this pair's K-proj: by the time the PE
                    # reaches them their qt pairs have arrived, while K-proj
                    # waits on this pair's kt DMA — in-order PE. Exception:
                    # at pr0, kt0 lands before qt0, so K-proj goes first.
                    avail = 8 * (1 + pr // 2)
                    target = min(len(f_units), avail, 2 + 4 * pr)
                    if pr == 0:
                        proj_pair(k_ps, wk_sb, kt_t, pr, range(NSB))
                    while fui < target:
                        emit_q_unit(*f_units[fui])
                        fui += 1
                    if pr > 0:
                        proj_pair(k_ps, wk_sb, kt_t, pr, range(NSB))
                for sb in range(NSB):
                    # Act, not DVE: keeps the F-end DVE queue clear for the
                    # V-A units' QTacc adds (their psum-reuse WAR gate)
                    nc.scalar.copy(KT_sb[:, sb * SB:(sb + 1) * SB], k_ps[sb][:])

                # ---- V projection in column halves so k-tiles 0..7 are
                # ---- ready at half-stream: Q's last block weaves into half 1
                # ---- (PE-idle DMA windows), attention qb0/qb1 into half 2.
                wv_sb = kvw.tile([128, NDC, 2, DK], F8, name="wv_sb", tag="kvw")
                nc.sync.dma_start(wv_sb[:, 0:BLK, :, :], wvT[:, 0:BLK, :, :])
                ident_sb = consts.tile([128, 128], BF16, name="ident_sb")
                masks_sb = consts.tile([128, 4 * SB], BF16, name="masks_sb")

                VT_sb = persist.tile([128, S], BF16, name="VT_sb")

                # --- attention machinery (emitted incrementally) ---
                hilo_tiles = {}
                wd_holder = []

                def attn_head(qb, h, mid=None):
                    # mid: emitted after the first score_exp — V-transpose
                    # groups slot here so their cross-engine latency hides
                    # behind this head's remaining scores
                    nkt = 4 * qb + 4  # causal: k-tiles 0..4qb+3
                    if qb not in hilo_tiles:
                        hilo_tiles[qb] = attnout.tile(
                            [128, NH_LOC, 2, SB], F8, name="hilo", tag="attn")
                    hilo = hilo_tiles[qb]
                    den_ps = psden.tile([128, SB], F32, name="den_ps", tag="den")
                    att_ps = pspv.tile([128, SB], F32, name="att_ps", tag="pv")
                    split = nkt <= 4
                    E_tiles = []

                    def score_exp(kt):
                        d = kt - 4 * qb
                        off = 128 * d if d >= 1 else 0
                        st_ps = ps.tile([128, SB], F32, name="st_ps", tag="ps")
                        nc.tensor.matmul(
                            st_ps[:, off:],
                            lhsT=KT_sb[:, kt * 128:(kt + 1) * 128],
                            rhs=QT_sb[:, h, qb * SB + off:(qb + 1) * SB],
                            start=True,
                            stop=True,
                        )
                        E1 = erot.tile([128, SB], BF16, name="E1", tag="E")
                        nc.scalar.activation(
                            E1[:, off:], st_ps[:, off:],
                            mybir.ActivationFunctionType.Exp,
                            scale=EXP_SCALE,
                        )
                        if d >= 0:
                            nc.vector.tensor_tensor(
                                E1[:, off:],
                                E1[:, off:],
                                masks_sb[:, d * SB + off:(d + 1) * SB],
                                mybir.AluOpType.mult,
                            )
                        return E1

                    def den_pv(kt, E1):
                        d = kt - 4 * qb
                        off = 128 * d if d >= 1 else 0
                        nc.tensor.matmul(
                            den_ps[:, off:],
                            lhsT=ones_sb[:, :],
                            rhs=E1[:, off:],
                            start=(kt == 0),
                            stop=(kt == nkt - 1),
                        )
                        nc.tensor.matmul(
                            att_ps[:, off:],
                            lhsT=V_sb[:, kt, :],
                            rhs=E1[:, off:],
                            start=(kt == 0),
                            stop=(kt == nkt - 1),
                        )

                    if split:
                        for kt in range(nkt):
                            E_tiles.append(score_exp(kt))
                            if kt == 0 and mid is not None:
                                mid()
                        for kt in range(nkt):
                            den_pv(kt, E_tiles[kt])
                    else:
                        # scores run one k-tile ahead of den/pv so the PE
                        # never waits out the exp+mask latency
                        E_prev = None
                        for kt in range(nkt):
                            E1 = score_exp(kt)
                            if kt == 0 and mid is not None:
                                mid()
                            if E_prev is not None:
                                den_pv(kt - 1, E_prev)
                            E_prev = E1
                        den_pv(nkt - 1, E_prev)
                    # normalize + split to fp8 hi/lo (attn scaled x16)
                    rec = small.tile([128, SB], F32, name="rec", tag="rec")
                    nc.vector.reciprocal(rec[:], den_ps[:])
                    abf = abfp.tile([128, SB], BF16, name="abf", tag="abf")
                    nc.vector.tensor_tensor(
                        abf[:], att_ps[:], rec[:], mybir.AluOpType.mult
                    )
                    nc.scalar.copy(hilo[:, h, 0, :], abf[:])
                    nc.vector.tensor_tensor(
                        hilo[:, h, 1, :], abf[:], hilo[:, h, 0, :],
                        mybir.AluOpType.subtract,
                    )
                    nc.sync.dma_start(
                        attn_loc[qb][h * 256:h * 256 + 256, :]
                        .rearrange("(hl p) q -> p hl q", p=128),
                        hilo[:, h, :, :],
                    )

                def attn_gather(qb):
                    if stage >= 4:
                        nc.gpsimd.collective_compute(
                            "AllGather",
                            mybir.AluOpType.bypass,
                            replica_groups=[list(range(N_CORES))],
                            ins=[attn_loc[qb][:]],
                            outs=[attn_gath[qb][:]],
                        )

                def wd_slice(qb):
                    if qb == 0:
                        wd_holder.append(bigw.tile(
                            [128, NDC, 2, FLOC], F8, name="wd_sb", tag="bigw"))
                    wdc = NDC // NSB
                    nc.sync.dma_start(
                        wd_holder[0][:, wdc * qb:wdc * (qb + 1), :, :],
                        wdT[:, wdc * qb:wdc * (qb + 1), :, :])

                def transp_group(g, pool, tag):
                    tp = pool.tile([128, 4, 128], BF16, name="tp", tag=tag)
                    for j in range(4):
                        st = 4 * g + j
                        nc.tensor.transpose(
                            tp[:, j, :], VT_sb[:, st * 128:(st + 1) * 128],
                            ident_sb[:])
                    nc.scalar.copy(V_sb[:, 4 * g:4 * (g + 1), :], tp[:])

                # --- V phase A (cols 0:512, sb0) — 12 Q units run here:
                # block 7's 8 plus block 6's deferred 4 (heads 2,3). Units are
                # emitted before the V-proj matmuls: their inputs (qt pairs)
                # land before vt does, so they fill the F-tail DMA window.
                # b6 units first (earliest-arriving qt, and each head's QTacc
                # chain must run b6 before b7).
                h1_units = [(NQB - 2, 2, 0), (NQB - 2, 2, 1),
                            (NQB - 2, 3, 0), (NQB - 2, 3, 1),
                            (NQB - 1, 0, 0), (NQB - 1, 0, 1),
                            (NQB - 1, 1, 0), (NQB - 1, 1, 1),
                            (NQB - 1, 2, 0), (NQB - 1, 2, 1),
                            (NQB - 1, 3, 0), (NQB - 1, 3, 1)]
                attn_jobs = [(0, 0), (0, 1), (0, 2), (0, 3),
                             (1, 0), (1, 1), (1, 2), (1, 3)]
                emitted = 0

                def pump_attn(n, mid=None):
                    nonlocal emitted
                    for _ in range(n):
                        if emitted >= len(attn_jobs):
                            if mid is not None:
                                mid()
                            return
                        qb, h = attn_jobs[emitted]
                        attn_head(qb, h, mid=mid)
                        mid = None
                        emitted += 1
                        if qb == 0 and h == NH_LOC - 1:
                            attn_gather(0)
                        if qb == 1 and h == NH_LOC - 1:
                            attn_gather(1)

                v_psA = ps.tile([128, SB], F32, name="v_psA", tag="ps")
                ui = 0
                for pr in range(NPR):
                    if pr % BPP == 1 and pr < NPR - BPP:
                        j = pr // BPP + 1
                        nc.sync.dma_start(
                            wv_sb[:, BLK * j:BLK * (j + 1), :, :],
                            wvT[:, BLK * j:BLK * (j + 1), :, :])
                    vt_t = vstream.tile([128, 2, 2, SB], F8,
                                        name="vt_t", tag="vt")
                    nc.sync.dma_start(vt_t[:], vT[:, pr, :, :, 0:SB])
                    if pr == 11:
                        nc.sync.dma_start(ident_sb[:], ident[:])
                        nc.sync.dma_start(masks_sb[:], masks[:])
                    if pr == 0:
                        while ui < 4:
                            emit_q_unit(*h1_units[ui])
                            ui += 1
                    elif ui < len(h1_units):
                        emit_q_unit(*h1_units[ui])
                        ui += 1
                    proj_pair([v_psA], wv_sb, vt_t, pr, [0])
                while ui < len(h1_units):
                    emit_q_unit(*h1_units[ui])
                    ui += 1
                nc.scalar.copy(VT_sb[:, 0:SB], v_psA[:])

                # --- V phase B (cols 512:1024, sb1) with qb0 woven ---
                v_psB = ps.tile([128, SB], F32, name="v_psB", tag="ps")
                for pr in range(NPR):
                    vt_t = vstream.tile([128, 2, 2, SB], F8,
                                        name="vt_t", tag="vt")
                    nc.sync.dma_start(vt_t[:], vT[:, pr, :, :, SB:2 * SB])
                    if stage >= 2 and pr % 4 == 0:
                        pump_attn(1, mid=(
                            (lambda: transp_group(0, pspv, "pv"))
                            if pr == 0 else None))
                    proj_pair([v_psB], wv_sb, vt_t, pr, [0])
                nc.scalar.copy(VT_sb[:, SB:2 * SB], v_psB[:])

                # --- V phase C (cols 1024:2048, sb2+sb3) with qb1 woven ---
                v_psC = [ps.tile([128, SB], F32, name=f"v_psC{i}", tag="ps")
                         for i in range(2)]
                for pr in range(NPR):
                    vt_t = vstream.tile([128, 2, 2, 2 * SB], F8,
                                        name="vt_t", tag="vt")
                    nc.sync.dma_start(vt_t[:], vT[:, pr, :, :, 2 * SB:4 * SB])
                    if stage >= 2 and pr % 4 == 0:
                        pump_attn(1, mid=(
                            (lambda: transp_group(1, psden, "den"))
                            if pr == 0 else None))
                    proj_pair(v_psC, wv_sb, vt_t, pr, range(2))
                for sb in range(2):
                    nc.scalar.copy(VT_sb[:, (2 + sb) * SB:(3 + sb) * SB],
                                   v_psC[sb][:])

                # ---- rest of attention (qb1 remainder, qb2, qb3) ----
                if stage >= 2:
                    pump_attn(len(attn_jobs) - emitted)
                    wd_slice(0)
                    wd_slice(1)
                    for qb in range(2, NSB):
                        for h in range(NH_LOC):
                            # V-transposes for tiles 8..15 slot behind qb2-h0's
                            # first score: its PV touches them only from
                            # k-tile 8 on, and the scores cover the VT-copy +
                            # transpose cross-engine latency
                            attn_head(qb, h, mid=(
                                (lambda: (transp_group(2, ps, "ps"),
                                          transp_group(3, ps, "ps")))
                                if (qb == 2 and h == 0) else None))
                        attn_gather(qb)
                        wd_slice(qb)

                # ---- output projection per q-block ----
                if stage >= 4:
                    def op_block(qb, dsubs):
                        o_ps = [
                            ps.tile([128, SB], F32, name=f"ops{d2}", tag="ps")
                            if d2 < 2 else
                            (pspv.tile([128, SB], F32, name=f"ops{d2}", tag="pv")
                             if d2 == 2 else
                             psden.tile([128, SB], F32, name=f"ops{d2}", tag="den"))
                            for d2 in dsubs
                        ]
                        for fc2 in range(NDC // 2):
                            at = atin.tile([128, 2, 2, SB], F8, name="at_c",
                                           tag="atin")
                            rowstart = fc2 * 512
                            nc.sync.dma_start(
                                at[:],
                                attn_gath[qb][rowstart:rowstart + 512, :]
                                .rearrange("(j hl p) q -> p j hl q", p=128, hl=2),
                            )
                            for j, dsub in enumerate(dsubs):
                                fcols = slice(dsub * 128, (dsub + 1) * 128)
                                nc.tensor.matmul(
                                    o_ps[j][:],
                                    lhsT=wd_holder[0][:, 2 * fc2:2 * fc2 + 2, 1, fcols],
                                    rhs=at[:, :, 0, :],
                                    start=(fc2 == 0), stop=False, perf_mode=DR,
                                )
                                for j2 in range(2):
                                    nc.tensor.matmul(
                                        o_ps[j][:],
                                        lhsT=wd_holder[0][:, 2 * fc2 + j2, :, fcols],
                                        rhs=at[:, j2, :, :],
                                        start=False,
                                        stop=(fc2 == NDC // 2 - 1 and j2 == 1),
                                        perf_mode=DR,
                                    )
                        # descale into one tile; two DMAs so the first pair's
                        # writeback overlaps the second pair's descale
                        o_all = osb.tile([128, 4, SB], BF16, name="o_all",
                                         tag="osb")
                        for j, dsub in enumerate(dsubs):
                            if j % 2 == 0:
                                nc.vector.tensor_scalar_mul(
                                    o_all[:, dsub, :], o_ps[j][:], OUT_DESCALE)
                            else:
                                nc.scalar.mul(
                                    o_all[:, dsub, :], o_ps[j][:], OUT_DESCALE)
                            if j == 1:
                                nc.sync.dma_start(
                                    outT[0:2 * 128, qb * SB:(qb + 1) * SB]
                                    .rearrange("(j p) q -> p j q", p=128),
                                    o_all[:, 0:2, :],
                                )
                        nc.sync.dma_start(
                            outT[2 * 128:FLOC, qb * SB:(qb + 1) * SB]
                            .rearrange("(j p) q -> p j q", p=128),
                            o_all[:, 2:4, :],
                        )

                    for qb in range(NSB):
                        op_block(qb, [0, 1, 2, 3])

            for rep in range(nrep):
                one_rep(rep)

    nc.compile()
    _legalize_dma_waits(nc)
    nc.codegen_inst_isa_subclasses()
    return nc


_NC_CACHE = None


def _get_nc():
    global _NC_CACHE
    if _NC_CACHE is None:
        _NC_CACHE = _build()
    return _NC_CACHE


def _split8(x, scale):
    """x (f32 [D, S]) -> hi, lo fp8 arrays of the scaled value."""
    f8 = ml_dtypes.float8_e4m3
    xs = x * np.float32(scale)
    hi = xs.astype(f8)
    lo = (xs - hi.astype(np.float32)).astype(f8)
    return hi, lo


def _act_layout(xT_hi, xT_lo):
    """[D, S] fp8 pair -> [128, NPR, 2, 2, S]."""
    h = xT_hi.reshape(NPR, 2, 128, S)
    l = xT_lo.reshape(NPR, 2, 128, S)
    out = np.empty((128, NPR, 2, 2, S), dtype=xT_hi.dtype)
    out[:, :, :, 0, :] = h.transpose(2, 0, 1, 3)
    out[:, :, :, 1, :] = l.transpose(2, 0, 1, 3)
    return np.ascontiguousarray(out)


def _w_layout(wT_hi, wT_lo):
    """[D, F] fp8 pair -> [128, NDC, 2(lo,hi), F]."""
    f = wT_hi.shape[1]
    h = wT_hi.reshape(NDC, 128, f)
    l = wT_lo.reshape(NDC, 128, f)
    out = np.empty((128, NDC, 2, f), dtype=wT_hi.dtype)
    out[:, :, 0, :] = l.transpose(1, 0, 2)
    out[:, :, 1, :] = h.transpose(1, 0, 2)
    return np.ascontiguousarray(out)


def _make_in_maps(q, k, v, Wq, Wk, Wv, Wd):
    bf = ml_dtypes.bfloat16
    scale = np.float32(DK) ** -0.5
    qT = np.ascontiguousarray(q.reshape(S, D).T)
    kT = np.ascontiguousarray(k.reshape(S, D).T)
    vT = np.ascontiguousarray(v.reshape(S, D).T)
    qA = _act_layout(*_split8(qT, S_A))
    kA = _act_layout(*_split8(kT, S_A))
    vA = _act_layout(*_split8(vT, S_A))

    kp = np.arange(128, dtype=np.int32)[:, None]
    qf = np.arange(SB, dtype=np.int32)[None, :]
    masks = np.concatenate(
        [(qf >= kp + 128 * d).astype(np.float32) for d in range(4)], axis=1
    ).astype(bf)
    ident = np.eye(128, dtype=np.float32).astype(bf)

    in_maps = []
    for c in range(N_CORES):
        fs = slice(FLOC * c, FLOC * (c + 1))
        ks = slice(DK * c, DK * (c + 1))
        in_maps.append({
            "qT": qA,
            "kT": kA,
            "vT": vA,
            "wqT": _w_layout(*_split8((Wq[fs, :] * scale).T, S_WQ)),
            "wkT": _w_layout(*_split8(Wk[ks, :].T, S_WKV)),
            "wvT": _w_layout(*_split8(Wv[ks, :].T, S_WKV)),
            "wdT": _w_layout(*_split8(Wd[fs, :].T, S_WD)),
            "masks": masks,
            "ident": ident,
        })
    return in_maps


def _assemble(results):
    outT_full = np.concatenate(
        [r["outT"].astype(np.float32) for r in results], axis=0)  # [4096, 2048]
    return np.ascontiguousarray(outT_full.T).reshape(1, S, D).astype(np.float32)


def kernel(q, k, v, Wq, Wk, Wv, Wd, _trace=False, **_ignored):
    nc = _get_nc()
    in_maps = _make_in_maps(
        np.asarray(q, np.float32), np.asarray(k, np.float32),
        np.asarray(v, np.float32), np.asarray(Wq, np.float32),
        np.asarray(Wk, np.float32), np.asarray(Wv, np.float32),
        np.asarray(Wd, np.float32),
    )
    res = run_bass_kernel_spmd(
        nc, in_maps, core_ids=list(range(N_CORES)), trace=_trace
    )
    out = _assemble(res.results)
    if _trace:
        return out, res
    return out


# revision 70
# speedup vs baseline: 43059.8467x; 1.0002x over previous
"""Trainium2 Bass kernel for GQA MultiHeadAttention (B=1, S=2048, D=4096,
H=32 query heads, HKV=8 kv heads, DK=DV=128), tensor-parallel across heads
on 8 NeuronCores.

Sharding: core c owns query heads 4c..4c+3 and kv head c (GQA group) and
computes its 512 attention features. The transposed attention output is
AllGathered across cores in four per-q-block collectives (hidden behind
later blocks' compute), then each core computes a 512-row slice of the
transposed output projection.

Schedule (PE ~88.6% busy; sim 330.4us vs 383.7us bf16 baseline):
- F: kt/qt chunk-pair streams + K-proj, with Q-proj "units" (one head x
  one sb-pair x one contraction block) woven in as their inputs land.
  Contraction blocks are variable-size pair-lists: a 1-pair block 0
  (units start ~7us in, filling the DMA-bound warmup) and a 1-pair
  block 8 whose short units defer into V phase A.
- V streams in three column phases A(sb0)/B(sb1)/C(sb2+3) so V k-tiles
  complete progressively: deferred Q units fill A; attention qb0 (plus
  the V-transposes, slotted mid-head behind its first score) weaves
  into B; qb1 into C. Each phase's PSUM->SBUF copies go to the Act
  engine to stay off the DVE's QTacc queue.
- Attention: fused score->exp->mask->den/pv per k-tile, scores running
  one k-tile ahead of den/pv to hide the exp+mask latency; diagonal
  tiles skip fully-masked leading columns (exact). Normalized attn is
  split to fp8 hi/lo (Act cast + DVE subtract) with rows (head,hl,p)
  so the out-proj consumer reads one contiguous block per fc pair.
- Out-proj per q-block; the last gather hides behind the first three
  blocks; all 4 dsubs descale into one tile with two output DMAs.

fp8 acceleration: all four projection GEMMs (Q/K/V/out) run as 3-term
hi/lo float8e4 splits on the PE's DoubleRow mode (2 stacked 128-deep
k-tiles per instruction at 0.5 cycles/col = 4x bf16 throughput):
    x @ W ~= x8@W8 (hi*hi, DoubleRow over chunk pairs)
           + [dW8@x8 + W8@dx8] (one DoubleRow per chunk, lo/hi stacked)
Weights and streamed activations are hi/lo-split and interleaved on the
host; the attention output is split on-device (Act cast + DVE subtract)
before the AllGather. Attention internals (scores/exp/mask/PV/den) stay
bf16: fp8 scores or probabilities fail the 2e-2 gate (measured 4.9e-2),
and raw exp() values overflow fp8's dynamic range.

Scales (powers of 2, exact): activations x16; Wq (with DK^-0.5 folded)
x2^13; Wk/Wv/Wd x2^9. The ones-matrix for the denominator is 512 so the
reciprocal folds the descale and attn comes out x16 ready for fp8; exp
descales scores by 2^-30 via the activation scale; output descales by
2^-13 into bf16 (host upcasts to f32).

Self-contained: hardcodes all shapes; inputs are the full unsharded tensors
keyed as in the problem's setup_inputs().
"""

import numpy as np
import ml_dtypes

import concourse.bacc as bacc
import concourse.mybir as mybir
from concourse.tile import TileContext
from concourse.bass_utils import run_bass_kernel_spmd

BF16 = mybir.dt.bfloat16
F8 = mybir.dt.float8e4
F32 = mybir.dt.float32
DR = mybir.MatmulPerfMode.DoubleRow

N_CORES = 8
S = 2048            # sequence length
D = 4096            # model dim
DK = 128            # head dim
NH_LOC = 4          # query heads per core
FLOC = NH_LOC * DK  # per-core attention features (512)
NDC = D // 128      # contraction chunks of 128 over D (32)
NPR = NDC // 2      # chunk pairs (16)
SB = 512            # q/s block width
NSB = S // SB       # 4
NST = S // 128      # 16 seq tiles of 128

S_A = 16.0          # activation scale
S_WQ = 2.0 ** 13    # Wq scale (DK^-0.5 folded first)
S_WKV = 2.0 ** 9
S_WD = 2.0 ** 9
ONES_VAL = 512.0    # den descale so attn psum*rec = 16*attn
EXP_SCALE = 2.0 ** -30
OUT_DESCALE = 2.0 ** -13

_DMA_TYPES = ("InstDMACopy", "InstDMATranspose")


def _legalize_dma_waits(nc):
    """DMA pseudo-instructions encode at most ONE sem wait (the ISA events
    slot). If Tile's sem assignment leaves more on a DMA, walrus rejects it
    ("Too many sync wait commands"). Hoist all but the last wait onto fresh
    nop instructions inserted immediately before the DMA on the same engine —
    the sequencer executes them in order, so semantics are identical."""
    ctr = 0
    for f in nc.m.functions:
        for blk in f.blocks:
            out = []
            changed = False
            for inst in blk.instructions:
                si = inst.sync_info
                if (
                    si is not None
                    and len(si.on_wait) > 1
                    and type(inst).__name__ in _DMA_TYPES
                ):
                    waits = list(si.on_wait)
                    for w in waits[:-1]:
                        nop = mybir.InstNoOp(
                            name=f"I-dmawaitfix-{ctr}", ins=[], outs=[]
                        )
                        ctr += 1
                        nop.engine = inst.engine
                        nop.sync_info = mybir.SyncInfo(on_wait=[w], on_update=[])
                        out.append(nop)
                    inst.sync_info = mybir.SyncInfo(
                        on_wait=[waits[-1]], on_update=list(si.on_update)
                    )
                    changed = True
                out.append(inst)
            if changed:
                blk.instructions = out
    return ctr


def _build(stage=4, nrep=1):
    nc = bacc.Bacc("TRN2", target_bir_lowering=False, num_devices=N_CORES,
                   dynamic_dma_scratch_size=2048)

    # ---- I/O ----
    # activations: [128, NPR, 2(chunk-in-pair), 2(hi,lo), S] fp8
    qT = nc.dram_tensor("qT", [128, NPR, 2, 2, S], F8, kind="ExternalInput")
    kT = nc.dram_tensor("kT", [128, NPR, 2, 2, S], F8, kind="ExternalInput")
    vT = nc.dram_tensor("vT", [128, NPR, 2, 2, S], F8, kind="ExternalInput")
    # weights: [128, NDC, 2(lo,hi), F] fp8 — hl order REVERSED vs
    # activations so one DoubleRow computes w_lo.T@x_hi + w_hi.T@x_lo
    wqT = nc.dram_tensor("wqT", [128, NDC, 2, FLOC], F8, kind="ExternalInput")
    wkT = nc.dram_tensor("wkT", [128, NDC, 2, DK], F8, kind="ExternalInput")
    wvT = nc.dram_tensor("wvT", [128, NDC, 2, DK], F8, kind="ExternalInput")
    wdT = nc.dram_tensor("wdT", [128, NDC, 2, FLOC], F8, kind="ExternalInput")
    masks = nc.dram_tensor("masks", [128, 4 * SB], BF16, kind="ExternalInput")
    ident = nc.dram_tensor("ident", [128, 128], BF16, kind="ExternalInput")
    outT = nc.dram_tensor("outT", [FLOC, S], BF16, kind="ExternalOutput")

    NBLK = 8
    BLK = NDC // NBLK  # 4 chunks (2 pairs) per Q contraction block
    BPP = BLK // 2     # pairs per block (2)

    with TileContext(nc) as tc:
        with (
            tc.tile_pool(name="consts", bufs=1) as consts,
            tc.tile_pool(name="kvw", bufs=1) as kvw,
            tc.tile_pool(name="bigw", bufs=1) as bigw,
            tc.tile_pool(name="persist", bufs=1) as persist,
            tc.tile_pool(name="qstream", bufs=4) as qstream,
            tc.tile_pool(name="kstream", bufs=3) as kstream,
            tc.tile_pool(name="vstream", bufs=6) as vstream,
            tc.tile_pool(name="vpre", bufs=2) as vpre,
            tc.tile_pool(name="erot", bufs=6) as erot,
            tc.tile_pool(name="small", bufs=1) as small,
            tc.tile_pool(name="attnout", bufs=2) as attnout,
            tc.tile_pool(name="abf", bufs=2) as abfp,
            tc.tile_pool(name="atin", bufs=6) as atin,
            tc.tile_pool(name="osb", bufs=2) as osb,
            tc.tile_pool(name="ps", bufs=4, space="PSUM") as ps,
            tc.tile_pool(name="pspv", bufs=2, space="PSUM") as pspv,
            tc.tile_pool(name="psden", bufs=2, space="PSUM") as psden,
            tc.tile_pool(name="dram", bufs=1, space="DRAM") as dram,
        ):
            def one_rep(rep):
                # wk first: K-proj's first matmul waits only on wk + kt[0]
                wk_sb = kvw.tile([128, NDC, 2, DK], F8, name="wk_sb", tag="kvw")
                nc.sync.dma_start(wk_sb[:, 0:BLK, :, :], wkT[:, 0:BLK, :, :])
                ones_sb = consts.tile([128, 128], BF16, name="ones_sb")
                nc.vector.memset(ones_sb[:], ONES_VAL)

                # persistent activations
                QT_sb = persist.tile([128, NH_LOC, S], BF16, name="QT_sb")
                KT_sb = persist.tile([128, S], BF16, name="KT_sb")
                V_sb = persist.tile([128, NST, DK], BF16, name="V_sb")
                QTacc = persist.tile([128, NH_LOC, S], BF16, name="QTacc")

                # per-q-block DRAM bounce buffers for the collectives
                # rows: (head, hl, p) so the consumer reads one contiguous
                # 512-row block per fc2 pair
                attn_loc = [
                    dram.tile([2 * FLOC, SB], F8, name=f"attn_loc{qb}", tag=f"al{qb}")
                    for qb in range(NSB)
                ]
                attn_gath = [
                    dram.tile([N_CORES * 2 * FLOC, SB], F8, name=f"attn_gath{qb}",
                              tag=f"ag{qb}", addr_space="Shared")
                    for qb in range(NSB)
                ]

                # ---- front: K-proj and Q-proj interleaved at pair level ----
                q_pairs = {}
                wq_holder = []

                # Q contraction blocks as pair-lists: a 1-pair block 0 (its
                # units start right after qt0 — fills the DMA-bound warmup)
                # and a 1-pair block 8 at the end (short units that defer
                # cheaply into V phase A).
                PAIR_BLOCKS = ([[0]] + [[2 * i + 1, 2 * i + 2]
                                        for i in range(7)] + [[15]])
                NQB = len(PAIR_BLOCKS)  # 9

                def emit_q_unit(b, f, pair):
                    # one head-feature (f) x one sb-pair of contraction block b
                    tpool, ttag = (pspv, "pv") if pair == 0 else (psden, "den")
                    qp = [
                        tpool.tile([128, SB], F32, name=f"qp{j}", tag=ttag)
                        for j in range(2)
                    ]
                    prs = PAIR_BLOCKS[b]
                    for pj, pr in enumerate(prs):
                        dc0 = 2 * pr
                        qt = q_pairs[pr]
                        for j2 in range(2):
                            s2 = 2 * pair + j2
                            cols = slice(s2 * SB, (s2 + 1) * SB)
                            fcols = slice(f * 128, (f + 1) * 128)
                            nc.tensor.matmul(
                                qp[j2][:],
                                lhsT=wq_holder[0][:, dc0:dc0 + 2, 1, fcols],
                                rhs=qt[:, :, 0, cols],
                                start=(pj == 0), stop=False, perf_mode=DR,
                            )
                            for j in range(2):
                                nc.tensor.matmul(
                                    qp[j2][:],
                                    lhsT=wq_holder[0][:, dc0 + j, :, fcols],
                                    rhs=qt[:, j, :, cols],
                                    start=False,
                                    stop=(pj == len(prs) - 1 and j == 1),
                                    perf_mode=DR,
                                )
                    for j in range(2):
                        s2 = 2 * pair + j
                        dst_acc = QTacc[:, f, s2 * SB:(s2 + 1) * SB]
                        if b == 0:
                            nc.vector.tensor_copy(dst_acc, qp[j][:])
                        elif b < NQB - 1:
                            nc.vector.tensor_tensor(
                                dst_acc, dst_acc, qp[j][:], mybir.AluOpType.add
                            )
                        else:
                            nc.vector.tensor_tensor(
                                QT_sb[:, f, s2 * SB:(s2 + 1) * SB],
                                dst_acc, qp[j][:], mybir.AluOpType.add,
                            )

                def proj_pair(ps_tiles, w_sb, x_t, pr, sbs):
                    # 3-term hi/lo DoubleRow for one chunk pair
                    dc0 = 2 * pr
                    for sb in sbs:
                        cols = slice(sb * SB, (sb + 1) * SB)
                        nc.tensor.matmul(
                            ps_tiles[sb][:],
                            lhsT=w_sb[:, dc0:dc0 + 2, 1, :],
                            rhs=x_t[:, :, 0, cols],
                            start=(pr == 0), stop=False, perf_mode=DR,
                        )
                        for j in range(2):
                            nc.tensor.matmul(
                                ps_tiles[sb][:],
                                lhsT=w_sb[:, dc0 + j, :, :],
                                rhs=x_t[:, j, :, cols],
                                start=False,
                                stop=(pr == NPR - 1 and j == 1),
                                perf_mode=DR,
                            )

                k_ps = [ps.tile([128, SB], F32, name=f"kps{i}", tag="ps")
                        for i in range(NSB)]
                # F emits blocks 0..7 minus block 7's heads 2,3 (those 4 and
                # all of block 8 defer into V phase A). Availability: block b
                # is runnable once its last qt pair (pr = 2b for b>=1, pr 0
                # for b0) and wq pair-slices have landed.
                f_units = [(b, f, pair) for b in range(NQB - 1)
                           for f in range(NH_LOC) for pair in range(2)][:-4]
                fui = 0
                for pr in range(NPR):
                    kt_t = kstream.tile([128, 2, 2, S], F8, name="kt_t",
                                        tag="kt")
                    nc.sync.dma_start(kt_t[:], kT[:, pr, :, :, :])
                    qt_t = qstream.tile([128, 2, 2, S], F8, name="qt_t",
                                        tag="qt")
                    nc.sync.dma_start(qt_t[:], qT[:, pr, :, :, :])
                    q_pairs[pr] = qt_t
                    if pr == 0:
                        wq_sb = bigw.tile([128, NDC, 2, FLOC], F8,
                                          name="wq_sb", tag="bigw")
                        wq_holder.append(wq_sb)
                    if pr % BPP == 0 and pr > 0:
                        dc0 = 2 * pr
                        nc.sync.dma_start(
                            wk_sb[:, dc0:dc0 + BLK, :, :],
                            wkT[:, dc0:dc0 + BLK, :, :])
                    # wq pair-slice (small, so block 0's units unblock early)
                    nc.sync.dma_start(
                        wq_sb[:, 2 * pr:2 * pr + 2, :, :],
                        wqT[:, 2 * pr:2 * pr + 2, :, :])
                    # units BEFORE this pair's K-proj: by the time the PE
                    # reaches them their qt pairs have arrived, while K-proj
                    # waits on this pair's kt DMA — in-order PE. Exception:
                    # at pr0, kt0 lands before qt0, so K-proj goes first.
                    avail = 8 * (1 + pr // 2)
                    target = min(len(f_units), avail, 2 + 4 * pr)
                    if pr == 0:
                        proj_pair(k_ps, wk_sb, kt_t, pr, range(NSB))
                    while fui < target:
                        emit_q_unit(*f_units[fui])
                        fui += 1
                    if pr > 0:
                        proj_pair(k_ps, wk_sb, kt_t, pr, range(NSB))
                for sb in range(NSB):
                    # Act, not DVE: keeps the F-end DVE queue clear for the
                    # V-A units' QTacc adds (their psum-reuse WAR gate)
                    nc.scalar.copy(KT_sb[:, sb * SB:(sb + 1) * SB], k_ps[sb][:])

                # ---- V projection in column halves so k-tiles 0..7 are
                # ---- ready at half-stream: Q's last block weaves into half 1
                # ---- (PE-idle DMA windows), attention qb0/qb1 into half 2.
                wv_sb = kvw.tile([128, NDC, 2, DK], F8, name="wv_sb", tag="kvw")
                nc.sync.dma_start(wv_sb[:, 0:BLK, :, :], wvT[:, 0:BLK, :, :])
                ident_sb = consts.tile([128, 128], BF16, name="ident_sb")
                masks_sb = consts.tile([128, 4 * SB], BF16, name="masks_sb")

                VT_sb = persist.tile([128, S], BF16, name="VT_sb")

                # --- attention machinery (emitted incrementally) ---
                hilo_tiles = {}
                wd_holder = []

                def attn_head(qb, h, mid=None):
                    # mid: emitted after the first score_exp — V-transpose
                    # groups slot here so their cross-engine latency hides
                    # behind this head's remaining scores
                    nkt = 4 * qb + 4  # causal: k-tiles 0..4qb+3
                    if qb not in hilo_tiles:
                        hilo_tiles[qb] = attnout.tile(
                            [128, NH_LOC, 2, SB], F8, name="hilo", tag="attn")
                    hilo = hilo_tiles[qb]
                    den_ps = psden.tile([128, SB], F32, name="den_ps", tag="den")
                    att_ps = pspv.tile([128, SB], F32, name="att_ps", tag="pv")
                    split = nkt <= 4
                    E_tiles = []

                    def score_exp(kt):
                        d = kt - 4 * qb
                        off = 128 * d if d >= 1 else 0
                        st_ps = ps.tile([128, SB], F32, name="st_ps", tag="ps")
                        nc.tensor.matmul(
                            st_ps[:, off:],
                            lhsT=KT_sb[:, kt * 128:(kt + 1) * 128],
                            rhs=QT_sb[:, h, qb * SB + off:(qb + 1) * SB],
                            start=True,
                            stop=True,
                        )
                        E1 = erot.tile([128, SB], BF16, name="E1", tag="E")
                        nc.scalar.activation(
                            E1[:, off:], st_ps[:, off:],
                            mybir.ActivationFunctionType.Exp,
                            scale=EXP_SCALE,
                        )
                        if d >= 0:
                            nc.vector.tensor_tensor(
                                E1[:, off:],
                                E1[:, off:],
                                masks_sb[:, d * SB + off:(d + 1) * SB],
                                mybir.AluOpType.mult,
                            )
                        return E1

                    def den_pv(kt, E1):
                        d = kt - 4 * qb
                        off = 128 * d if d >= 1 else 0
                        nc.tensor.matmul(
                            den_ps[:, off:],
                            lhsT=ones_sb[:, :],
                            rhs=E1[:, off:],
                            start=(kt == 0),
                            stop=(kt == nkt - 1),
                        )
                        nc.tensor.matmul(
                            att_ps[:, off:],
                            lhsT=V_sb[:, kt, :],
                            rhs=E1[:, off:],
                            start=(kt == 0),
                            stop=(kt == nkt - 1),
                        )

                    if split:
                        for kt in range(nkt):
                            E_tiles.append(score_exp(kt))
                            if kt == 0 and mid is not None:
                                mid()
                        for kt in range(nkt):
                            den_pv(kt, E_tiles[kt])
                    else:
                        # scores run one k-tile ahead of den/pv so the PE
                        # never waits out the exp+mask latency
                        E_prev = None
                        for kt in range(nkt):
                            E1 = score_exp(kt)
                            if kt == 0 and mid is not None:
                                mid()
                            if E_prev is not None:
                                den_pv(kt - 1, E_prev)
                            E_prev = E1
                        den_pv(nkt - 1, E_prev)
                    # normalize + split to fp8 hi/lo (attn scaled x16)
                    rec = small.tile([128, SB], F32, name="rec", tag="rec")
                    nc.vector.reciprocal(rec[:], den_ps[:])
                    abf = abfp.tile([128, SB], BF16, name="abf", tag="abf")
                    nc.vector.tensor_tensor(
                        abf[:], att_ps[:], rec[:], mybir.AluOpType.mult
                    )
                    nc.scalar.copy(hilo[:, h, 0, :], abf[:])
                    nc.vector.tensor_tensor(
                        hilo[:, h, 1, :], abf[:], hilo[:, h, 0, :],
                        mybir.AluOpType.subtract,
                    )
                    nc.sync.dma_start(
                        attn_loc[qb][h * 256:h * 256 + 256, :]
                        .rearrange("(hl p) q -> p hl q", p=128),
                        hilo[:, h, :, :],
                    )

                def attn_gather(qb):
                    if stage >= 4:
                        nc.gpsimd.collective_compute(
                            "AllGather",
                            mybir.AluOpType.bypass,
                            replica_groups=[list(range(N_CORES))],
                            ins=[attn_loc[qb][:]],
                            outs=[attn_gath[qb][:]],
                        )

                def wd_slice(qb):
                    if qb == 0:
                        wd_holder.append(bigw.tile(
                            [128, NDC, 2, FLOC], F8, name="wd_sb", tag="bigw"))
                    wdc = NDC // NSB
                    nc.sync.dma_start(
                        wd_holder[0][:, wdc * qb:wdc * (qb + 1), :, :],
                        wdT[:, wdc * qb:wdc * (qb + 1), :, :])

                def transp_group(g, pool, tag):
                    tp = pool.tile([128, 4, 128], BF16, name="tp", tag=tag)
                    for j in range(4):
                        st = 4 * g + j
                        nc.tensor.transpose(
                            tp[:, j, :], VT_sb[:, st * 128:(st + 1) * 128],
                            ident_sb[:])
                    nc.scalar.copy(V_sb[:, 4 * g:4 * (g + 1), :], tp[:])

                # --- V phase A (cols 0:512, sb0) — 12 Q units run here:
                # block 7's 8 plus block 6's deferred 4 (heads 2,3). Units are
                # emitted before the V-proj matmuls: their inputs (qt pairs)
                # land before vt does, so they fill the F-tail DMA window.
                # b6 units first (earliest-arriving qt, and each head's QTacc
                # chain must run b6 before b7).
                h1_units = [(NQB - 2, 2, 0), (NQB - 2, 2, 1),
                            (NQB - 2, 3, 0), (NQB - 2, 3, 1),
                            (NQB - 1, 0, 0), (NQB - 1, 0, 1),
                            (NQB - 1, 1, 0), (NQB - 1, 1, 1),
                            (NQB - 1, 2, 0), (NQB - 1, 2, 1),
                            (NQB - 1, 3, 0), (NQB - 1, 3, 1)]
                attn_jobs = [(0, 0), (0, 1), (0, 2), (0, 3),
                             (1, 0), (1, 1), (1, 2), (1, 3)]
                emitted = 0

                def pump_attn(n, mid=None):
                    nonlocal emitted
                    for _ in range(n):
                        if emitted >= len(attn_jobs):
                            if mid is not None:
                                mid()
                            return
                        qb, h = attn_jobs[emitted]
                        attn_head(qb, h, mid=mid)
                        mid = None
                        emitted += 1
                        if qb == 0 and h == NH_LOC - 1:
                            attn_gather(0)
                        if qb == 1 and h == NH_LOC - 1:
                            attn_gather(1)

                v_psA = ps.tile([128, SB], F32, name="v_psA", tag="ps")
                b_pre = []
                ui = 0
                for pr in range(NPR):
                    if pr in (9, 10):
                        # B's first two tiles prefetch from a dedicated pool
                        # (no vstream slot contention); A is PE-bound so the
                        # DMA-queue displacement is free
                        bt = vpre.tile([128, 2, 2, SB], F8, name="vpre_t",
                                       tag="vpre")
                        nc.sync.dma_start(bt[:], vT[:, pr - 9, :, :, SB:2 * SB])
                        b_pre.append(bt)
                    if pr % BPP == 1 and pr < NPR - BPP:
                        j = pr // BPP + 1
                        nc.sync.dma_start(
                            wv_sb[:, BLK * j:BLK * (j + 1), :, :],
                            wvT[:, BLK * j:BLK * (j + 1), :, :])
                    vt_t = vstream.tile([128, 2, 2, SB], F8,
                                        name="vt_t", tag="vt")
                    nc.sync.dma_start(vt_t[:], vT[:, pr, :, :, 0:SB])
                    if pr == 11:
                        nc.sync.dma_start(ident_sb[:], ident[:])
                        nc.sync.dma_start(masks_sb[:], masks[:])
                    if pr == 0:
                        while ui < 4:
                            emit_q_unit(*h1_units[ui])
                            ui += 1
                    elif ui < len(h1_units):
                        emit_q_unit(*h1_units[ui])
                        ui += 1
                    proj_pair([v_psA], wv_sb, vt_t, pr, [0])
                while ui < len(h1_units):
                    emit_q_unit(*h1_units[ui])
                    ui += 1
                nc.scalar.copy(VT_sb[:, 0:SB], v_psA[:])

                # --- V phase B (cols 512:1024, sb1) with qb0 woven ---
                v_psB = ps.tile([128, SB], F32, name="v_psB", tag="ps")
                for pr in range(NPR):
                    if pr < 2:
                        vt_t = b_pre[pr]
                    else:
                        vt_t = vstream.tile([128, 2, 2, SB], F8,
                                            name="vt_t", tag="vt")
                        nc.sync.dma_start(vt_t[:], vT[:, pr, :, :, SB:2 * SB])
                    if stage >= 2 and pr % 4 == 0:
                        pump_attn(1, mid=(
                            (lambda: transp_group(0, pspv, "pv"))
                            if pr == 0 else None))
                    proj_pair([v_psB], wv_sb, vt_t, pr, [0])
                nc.scalar.copy(VT_sb[:, SB:2 * SB], v_psB[:])

                # --- V phase C (cols 1024:2048, sb2+sb3) with qb1 woven ---
                v_psC = [ps.tile([128, SB], F32, name=f"v_psC{i}", tag="ps")
                         for i in range(2)]
                for pr in range(NPR):
                    vt_t = vstream.tile([128, 2, 2, 2 * SB], F8,
                                        name="vt_t", tag="vt")
                    nc.sync.dma_start(vt_t[:], vT[:, pr, :, :, 2 * SB:4 * SB])
                    if stage >= 2 and pr % 4 == 0:
                        pump_attn(1, mid=(
                            (lambda: transp_group(1, psden, "den"))
                            if pr == 0 else None))
                    proj_pair(v_psC, wv_sb, vt_t, pr, range(2))
                for sb in range(2):
                    nc.scalar.copy(VT_sb[:, (2 + sb) * SB:(3 + sb) * SB],
                                   v_psC[sb][:])

                # ---- rest of attention (qb1 remainder, qb2, qb3) ----
                if stage >= 2:
                    pump_attn(len(attn_jobs) - emitted)
                    wd_slice(0)
                    wd_slice(1)
                    for qb in range(2, NSB):
                        for h in range(NH_LOC):
                            # V-transposes for tiles 8..15 slot behind qb2-h0's
                            # first score: its PV touches them only from
                            # k-tile 8 on, and the scores cover the VT-copy +
                            # transpose cross-engine latency
                            attn_head(qb, h, mid=(
                                (lambda: (transp_group(2, ps, "ps"),
                                          transp_group(3, ps, "ps")))
                                if (qb == 2 and h == 0) else None))
                        attn_gather(qb)
                        wd_slice(qb)

                # ---- output projection per q-block ----
                if stage >= 4:
                    def op_block(qb, dsubs):
                        o_ps = [
                            ps.tile([128, SB], F32, name=f"ops{d2}", tag="ps")
                            if d2 < 2 else
                            (pspv.tile([128, SB], F32, name=f"ops{d2}", tag="pv")
                             if d2 == 2 else
                             psden.tile([128, SB], F32, name=f"ops{d2}", tag="den"))
                            for d2 in dsubs
                        ]
                        for fc2 in range(NDC // 2):
                            at = atin.tile([128, 2, 2, SB], F8, name="at_c",
                                           tag="atin")
                            rowstart = fc2 * 512
                            nc.sync.dma_start(
                                at[:],
                                attn_gath[qb][rowstart:rowstart + 512, :]
                                .rearrange("(j hl p) q -> p j hl q", p=128, hl=2),
                            )
                            for j, dsub in enumerate(dsubs):
                                fcols = slice(dsub * 128, (dsub + 1) * 128)
                                nc.tensor.matmul(
                                    o_ps[j][:],
                                    lhsT=wd_holder[0][:, 2 * fc2:2 * fc2 + 2, 1, fcols],
                                    rhs=at[:, :, 0, :],
                                    start=(fc2 == 0), stop=False, perf_mode=DR,
                                )
                                for j2 in range(2):
                                    nc.tensor.matmul(
                                        o_ps[j][:],
                                        lhsT=wd_holder[0][:, 2 * fc2 + j2, :, fcols],
                                        rhs=at[:, j2, :, :],
                                        start=False,
                                        stop=(fc2 == NDC // 2 - 1 and j2 == 1),
                                        perf_mode=DR,
                                    )
                        # descale into one tile; two DMAs so the first pair's
                        # writeback overlaps the second pair's descale
                        o_all = osb.tile([128, 4, SB], BF16, name="o_all",
                                         tag="osb")
                        for j, dsub in enumerate(dsubs):
                            if j % 2 == 0:
                                nc.vector.tensor_scalar_mul(
                                    o_all[:, dsub, :], o_ps[j][:], OUT_DESCALE)
                            else:
                                nc.scalar.mul(
                                    o_all[:, dsub, :], o_ps[j][:], OUT_DESCALE)
                            if j == 1:
                                nc.sync.dma_start(
                                    outT[0:2 * 128, qb * SB:(qb + 1) * SB]
                                    .rearrange("(j p) q -> p j q", p=128),
                                    o_all[:, 0:2, :],
                                )
                        nc.sync.dma_start(
                            outT[2 * 128:FLOC, qb * SB:(qb + 1) * SB]
                            .rearrange("(j p) q -> p j q", p=128),
                            o_all[:, 2:4, :],
                        )

                    for qb in range(NSB):
                        op_block(qb, [0, 1, 2, 3])

            for rep in range(nrep):
                one_rep(rep)

    nc.compile()
    _legalize_dma_waits(nc)
    nc.codegen_inst_isa_subclasses()
    return nc


_NC_CACHE = None


def _get_nc():
    global _NC_CACHE
    if _NC_CACHE is None:
        _NC_CACHE = _build()
    return _NC_CACHE


def _split8(x, scale):
    """x (f32 [D, S]) -> hi, lo fp8 arrays of the scaled value."""
    f8 = ml_dtypes.float8_e4m3
    xs = x * np.float32(scale)
    hi = xs.astype(f8)
    lo = (xs - hi.astype(np.float32)).astype(f8)
    return hi, lo


def _act_layout(xT_hi, xT_lo):
    """[D, S] fp8 pair -> [128, NPR, 2, 2, S]."""
    h = xT_hi.reshape(NPR, 2, 128, S)
    l = xT_lo.reshape(NPR, 2, 128, S)
    out = np.empty((128, NPR, 2, 2, S), dtype=xT_hi.dtype)
    out[:, :, :, 0, :] = h.transpose(2, 0, 1, 3)
    out[:, :, :, 1, :] = l.transpose(2, 0, 1, 3)
    return np.ascontiguousarray(out)


def _w_layout(wT_hi, wT_lo):
    """[D, F] fp8 pair -> [128, NDC, 2(lo,hi), F]."""
    f = wT_hi.shape[1]
    h = wT_hi.reshape(NDC, 128, f)
    l = wT_lo.reshape(NDC, 128, f)
    out = np.empty((128, NDC, 2, f), dtype=wT_hi.dtype)
    out[:, :, 0, :] = l.transpose(1, 0, 2)
    out[:, :, 1, :] = h.transpose(1, 0, 2)
    return np.ascontiguousarray(out)


def _make_in_maps(q, k, v, Wq, Wk, Wv, Wd):
    bf = ml_dtypes.bfloat16
    scale = np.float32(DK) ** -0.5
    qT = np.ascontiguousarray(q.reshape(S, D).T)
    kT = np.ascontiguousarray(k.reshape(S, D).T)
    vT = np.ascontiguousarray(v.reshape(S, D).T)
    qA = _act_layout(*_split8(qT, S_A))
    kA = _act_layout(*_split8(kT, S_A))
    vA = _act_layout(*_split8(vT, S_A))

    kp = np.arange(128, dtype=np.int32)[:, None]
    qf = np.arange(SB, dtype=np.int32)[None, :]
    masks = np.concatenate(
        [(qf >= kp + 128 * d).astype(np.float32) for d in range(4)], axis=1
    ).astype(bf)
    ident = np.eye(128, dtype=np.float32).astype(bf)

    in_maps = []
    for c in range(N_CORES):
        fs = slice(FLOC * c, FLOC * (c + 1))
        ks = slice(DK * c, DK * (c + 1))
        in_maps.append({
            "qT": qA,
            "kT": kA,
            "vT": vA,
            "wqT": _w_layout(*_split8((Wq[fs, :] * scale).T, S_WQ)),
            "wkT": _w_layout(*_split8(Wk[ks, :].T, S_WKV)),
            "wvT": _w_layout(*_split8(Wv[ks, :].T, S_WKV)),
            "wdT": _w_layout(*_split8(Wd[fs, :].T, S_WD)),
            "masks": masks,
            "ident": ident,
        })
    return in_maps


def _assemble(results):
    outT_full = np.concatenate(
        [r["outT"].astype(np.float32) for r in results], axis=0)  # [4096, 2048]
    return np.ascontiguousarray(outT_full.T).reshape(1, S, D).astype(np.float32)


def kernel(q, k, v, Wq, Wk, Wv, Wd, _trace=False, **_ignored):
    nc = _get_nc()
    in_maps = _make_in_maps(
        np.asarray(q, np.float32), np.asarray(k, np.float32),
        np.asarray(v, np.float32), np.asarray(Wq, np.float32),
        np.asarray(Wk, np.float32), np.asarray(Wv, np.float32),
        np.asarray(Wd, np.float32),
    )
    res = run_bass_kernel_spmd(
        nc, in_maps, core_ids=list(range(N_CORES)), trace=_trace
    )
    out = _assemble(res.results)
    if _trace:
        return out, res
    return out
